# revision 6
# baseline (speedup 1.0000x reference)
"""DeepSeekMoE (B=2,S=2048,H=1024,I=2816, 7 routed experts top-2 + 1 shared) on 8 trn2 NeuronCores.

Strategy: collective-free unified expert-parallel.
  The shared expert has the same architecture as the routed experts, so every
  unit of work is "one MLP applied to one column" — a column is either a
  (token, routed-expert) slot or a (token, shared) slot.  12288 slot-columns
  total are packed into 8 cores x 2 uniform segments:
    seg1 (cap c1 = max expert load): core c < 7 carries routed expert c's
      entire token list; the remaining seg1 slots are shared-token filler.
    seg2 (cap c2): shared-token filler on every core.
  The top-2 combine weight is folded into the up-projection input on the host
  (xw = w * x), so slot outputs need no on-chip scaling, no scatter and no
  ReduceScatter: each core returns yt = down(silu(xg@G) * (xw@U)) [H, C] and
  the host sums each token's 3 slots (shared + 2 routed) — O(T*H) adds,
  ~0.01% of the FLOPs, same spirit as the host router.

  Device schedule per core (all matmuls bf16, f32 psum):
    1. gate/up: 22 I-chunks; per chunk stream 4 weight tiles (g/u x 2 segs)
       and run K=8-deep matmul groups over ~512-col tiles; silu on the scalar
       engine; h = silu(g)*u written to HT (bf16) by the vector engine.
    2. down: 8 H-chunks; per chunk stream 2 down-weight tiles (2 segs),
       K=22-deep matmul groups over the same column tiles, psum copied out on
       the scalar engine and DMAed to yt [H, C] f32.
  No collectives -> DMA streams freely; weights stream (2 expert sets/core,
  ~35 MB) far below the ~120 GB/s needed to keep pace with the PE.
"""

import math
import os
import sys
import types

import numpy as np
import ml_dtypes

for _p in ('/opt/trn_rl_repo', '/root/.axon_site/_ro/trn_rl_repo'):
    if os.path.isdir(_p) and _p not in sys.path:
        sys.path.append(_p)


def _install_profile_glue():
    """Optional: register the NTFF profile hook so trace=True/BASS_TRACE works
    under axon (the image's antenv lacks axon_hooks). Harmless if unavailable."""
    try:
        import antenv
        if 'antenv.axon_hooks' in sys.modules:
            return
        mod = types.ModuleType('antenv.axon_hooks')
        holder = [None]
        mod.set_axon_ntff_profile_hook = lambda h: holder.__setitem__(0, h)
        mod.get_axon_ntff_profile_hook = lambda: holder[0]
        sys.modules['antenv.axon_hooks'] = mod
        antenv.axon_hooks = mod
        so = '/opt/axon/libaxon_pjrt.so'
        if os.path.exists(so):
            from trn_agent_boot.trn_boot import _ntff_profile_via_ctypes
            hook = _ntff_profile_via_ctypes(so)
            if hook is not None:
                mod.set_axon_ntff_profile_hook(hook)
    except Exception:
        pass


_install_profile_glue()

import concourse.bass as bass
import concourse.mybir as mybir
from concourse.bass_utils import run_bass_kernel_spmd
from concourse.tile import TileContext

B, S, H, I = 2, 2048, 1024, 2816
E_ROUTED = 7
TOP_K = 2
T = B * S                  # 4096 tokens
NCORES = 8
KH = H // 128              # 8 contraction chunks over H (gate/up) = output chunks (down)
KI = I // 128              # 22 contraction chunks over I (down) = output chunks (gate/up)

F32 = mybir.dt.float32
BF16 = mybir.dt.bfloat16

LAST_RESULT = None         # BassKernelResults of the most recent run (for tests)

_PROG_CACHE = {}


def _split_sync_waits(nc, max_waits=1):
    """This container's walrus rejects >1 sync wait per instruction; spill
    extra waits onto same-engine NoOps placed just before the instruction."""
    for f in nc.m.functions:
        for bb in f.blocks:
            new_list = []
            changed = False
            for inst in bb.instructions:
                si = inst.sync_info
                if si is not None and si.on_wait is not None and len(si.on_wait) > max_waits:
                    waits = list(si.on_wait)
                    while len(waits) > max_waits:
                        chunk, waits = waits[:max_waits], waits[max_waits:]
                        nop = mybir.InstNoOp(
                            name=nc.get_next_instruction_name(),
                            engine=inst.engine, bass_nofuse=True,
                            sync_info=mybir.SyncInfo(on_wait=chunk, on_update=[]),
                        )
                        new_list.append(nop)
                    inst.sync_info = mybir.SyncInfo(
                        on_wait=waits, on_update=list(si.on_update or []))
                    changed = True
                new_list.append(inst)
            if changed:
                bb.instructions[:] = new_list


def _even_tiles(offset, total, width=512):
    """Split [offset, offset+total) into near-even tiles of <= width cols."""
    if total <= 0:
        return []
    n = (total + width - 1) // width
    base, rem = divmod(total, n)
    out = []
    c = offset
    for j in range(n):
        tn = base + (1 if j < rem else 0)
        out.append((c, tn))
        c += tn
    return out


def _build_program(caps):
    """Uniform SPMD program for segment capacities (c1, c2)."""
    c1, c2 = caps
    C = c1 + c2
    tiles1 = _even_tiles(0, c1)
    tiles2 = _even_tiles(c1, c2)

    nc = bass.Bass()
    xg = nc.declare_dram_parameter('xg', [H, C], BF16, isOutput=False)
    xw = nc.declare_dram_parameter('xw', [H, C], BF16, isOutput=False)
    # gate/up weights arrive chunk-shuffled: [KI, 128, KH, 128] so each
    # per-I-chunk stream DMA reads 2KB-contiguous per partition.
    g1 = nc.declare_dram_parameter('g1', [KI, 128, KH, 128], BF16, isOutput=False)
    u1 = nc.declare_dram_parameter('u1', [KI, 128, KH, 128], BF16, isOutput=False)
    g2 = nc.declare_dram_parameter('g2', [KI, 128, KH, 128], BF16, isOutput=False)
    u2 = nc.declare_dram_parameter('u2', [KI, 128, KH, 128], BF16, isOutput=False)
    # down weights shuffled per output H-chunk: [KH, 128, KI, 128]
    d1 = nc.declare_dram_parameter('d1', [KH, 128, KI, 128], BF16, isOutput=False)
    d2 = nc.declare_dram_parameter('d2', [KH, 128, KI, 128], BF16, isOutput=False)
    yt = nc.declare_dram_parameter('yt', [H, C], F32, isOutput=True)

    xg_r = xg.rearrange('(k p) c -> p k c', p=128)
    xw_r = xw.rearrange('(k p) c -> p k c', p=128)
    warm_out = nc.dram_tensor('warm', [128, 128], F32)

    with TileContext(nc) as tc:
        with (
            tc.tile_pool(name='big', bufs=1) as bigp,
            tc.tile_pool(name='wstream', bufs=2) as wsp,
            tc.tile_pool(name='dstream', bufs=2) as dsp,
            tc.tile_pool(name='rtp', bufs=3) as rtp,
            tc.tile_pool(name='stg', bufs=3) as stg,
            tc.tile_pool(name='ps', bufs=8, space='PSUM') as psp,
        ):
            XG = bigp.tile([128, KH, C], BF16, tag='XG')
            XW = bigp.tile([128, KH, C], BF16, tag='XW')
            HT = bigp.tile([128, KI, C], BF16, tag='HT')

            # ---- 0. PE clock warmup during input staging: the tensor engine
            # ramps to max frequency only after ~3us of continuous execution,
            # so burn dummy matmuls while the first DMAs land.
            wu = bigp.tile([128, 256], BF16, tag='wu')
            nc.vector.memset(wu[:, :], 0.0)
            wps = psp.tile([128, 512], F32, tag='ps', name='warm')
            WARM = 28
            for r in range(WARM):
                nc.tensor.matmul(wps[:, :256], lhsT=wu[:, :128],
                                 rhs=wu[:, :],
                                 start=(r == 0), stop=(r == WARM - 1))
            wst = stg.tile([128, 512], F32, tag='yst', name='warmst')
            nc.scalar.copy(out=wst[:, :128], in_=wps[:, :128])
            nc.sync.dma_start(out=warm_out[:, :], in_=wst[:, :128])

            # ---- 1. gate/up over 22 I-chunks
            for i in range(KI):
                gch1 = wsp.tile([128, KH, 128], BF16, tag='g1', name=f'g1_{i}')
                nc.sync.dma_start(out=gch1[:, :, :], in_=g1[i, :, :, :])
                gch2 = wsp.tile([128, KH, 128], BF16, tag='g2', name=f'g2_{i}')
                nc.sync.dma_start(out=gch2[:, :, :], in_=g2[i, :, :, :])
                if i == 0:
                    # tile-granular staging so the first matmul group only
                    # waits on its own ~0.8MB of x, not the full 3.2MB
                    for (t0, tn) in tiles1 + tiles2:
                        for k in range(KH):
                            nc.sync.dma_start(
                                out=XG[:, k, t0:t0 + tn], in_=xg_r[:, k, t0:t0 + tn])
                uch1 = wsp.tile([128, KH, 128], BF16, tag='u1', name=f'u1_{i}')
                nc.sync.dma_start(out=uch1[:, :, :], in_=u1[i, :, :, :])
                uch2 = wsp.tile([128, KH, 128], BF16, tag='u2', name=f'u2_{i}')
                nc.sync.dma_start(out=uch2[:, :, :], in_=u2[i, :, :, :])
                if i == 0:
                    for (t0, tn) in tiles1 + tiles2:
                        for k in range(KH):
                            nc.sync.dma_start(
                                out=XW[:, k, t0:t0 + tn], in_=xw_r[:, k, t0:t0 + tn])
                for (gch, uch, tiles) in ((gch1, uch1, tiles1), (gch2, uch2, tiles2)):
                    for (t0, tn) in tiles:
                        gps = psp.tile([128, 512], F32, tag='ps', name=f'g{i}_{t0}')
                        for k in range(KH):
                            nc.tensor.matmul(
                                gps[:, :tn], lhsT=gch[:, k, :],
                                rhs=XG[:, k, t0:t0 + tn],
                                start=(k == 0), stop=(k == KH - 1))
                        at = rtp.tile([128, 512], F32, tag='at', name=f'at{i}_{t0}')
                        nc.scalar.activation(
                            out=at[:, :tn], in_=gps[:, :tn],
                            func=mybir.ActivationFunctionType.Silu)
                        ups = psp.tile([128, 512], F32, tag='ps', name=f'u{i}_{t0}')
                        for k in range(KH):
                            nc.tensor.matmul(
                                ups[:, :tn], lhsT=uch[:, k, :],
                                rhs=XW[:, k, t0:t0 + tn],
                                start=(k == 0), stop=(k == KH - 1))
                        nc.vector.tensor_tensor(
                            out=HT[:, i, t0:t0 + tn],
                            in0=at[:, :tn], in1=ups[:, :tn],
                            op=mybir.AluOpType.mult)

            # ---- 2. down over 8 H-chunks
            for h in range(KH):
                dch1 = dsp.tile([128, KI, 128], BF16, tag='d1', name=f'd1_{h}')
                nc.sync.dma_start(out=dch1[:, :, :], in_=d1[h, :, :, :])
                dch2 = dsp.tile([128, KI, 128], BF16, tag='d2', name=f'd2_{h}')
                nc.sync.dma_start(out=dch2[:, :, :], in_=d2[h, :, :, :])
                for (dch, tiles) in ((dch1, tiles1), (dch2, tiles2)):
                    for (t0, tn) in tiles:
                        yps = psp.tile([128, 512], F32, tag='ps', name=f'y{h}_{t0}')
                        for k in range(KI):
                            nc.tensor.matmul(
                                yps[:, :tn], lhsT=dch[:, k, :],
                                rhs=HT[:, k, t0:t0 + tn],
                                start=(k == 0), stop=(k == KI - 1))
                        yst = stg.tile([128, 512], F32, tag='yst', name=f'ys{h}_{t0}')
                        nc.scalar.copy(out=yst[:, :tn], in_=yps[:, :tn])
                        nc.sync.dma_start(
                            out=yt[h * 128:(h + 1) * 128, t0:t0 + tn],
                            in_=yst[:, :tn])

    _split_sync_waits(nc)
    return nc


def _dispatch(x2, router_w, routing_bias):
    """Host router. Returns per-expert token lists [(token, weight)...]."""
    logits = x2 @ router_w + routing_bias            # [T, 7] fp32
    order = np.argsort(-logits, axis=1, kind='stable')[:, :TOP_K]
    probs = 1.0 / (1.0 + np.exp(-logits))
    rows = np.arange(T)
    s = probs[rows[:, None], order]                  # [T, 2]
    w = s / s.sum(axis=1, keepdims=True)             # renormalized combine weights

    lists = [[] for _ in range(E_ROUTED)]
    for k in range(TOP_K):
        for t, e, wt in zip(rows, order[:, k], w[:, k]):
            lists[e].append((int(t), float(wt)))
    return lists


def _shuffle_gateup(wmat):
    """[H, I] -> [KI, 128(H-part), KH, 128(I-cols)] bf16."""
    return np.ascontiguousarray(
        wmat.reshape(KH, 128, KI, 128).transpose(2, 1, 0, 3).astype(ml_dtypes.bfloat16))


def _shuffle_down(wmat):
    """[I, H] -> [KH(h), 128(I-part), KI(k), 128(H-cols)] bf16."""
    return np.ascontiguousarray(
        wmat.reshape(KI, 128, KH, 128).transpose(2, 1, 0, 3).astype(ml_dtypes.bfloat16))


def kernel(x, router_w, routing_bias, shared_gate, shared_up, shared_down,
           routed_gate, routed_up, routed_down):
    global LAST_RESULT
    x = np.asarray(x, np.float32)
    x2 = x.reshape(T, H)

    lists = _dispatch(x2, np.asarray(router_w, np.float32),
                      np.asarray(routing_bias, np.float32))

    # pieces: split any oversized expert so every piece fits one seg1 slot
    pieces = []                       # (expert_id, [(token, weight)...])
    for e in range(E_ROUTED):
        le = lists[e]
        nsplit = max(1, (len(le) + 2047) // 2048)
        step = (len(le) + nsplit - 1) // nsplit
        for a in range(0, len(le), step):
            pieces.append((e, le[a:a + step]))
    assert len(pieces) <= NCORES, 'expert pieces exceed core count'
    c1 = max(128, max(len(toks) for _, toks in pieces))
    n_spare = NCORES - len(pieces)
    c2 = max(0, -(-(T - n_spare * c1) // NCORES))
    c2 = max(c2, 1)
    C = c1 + c2

    bf = ml_dtypes.bfloat16
    routed_gate = np.asarray(routed_gate, np.float32)
    routed_up = np.asarray(routed_up, np.float32)
    routed_down = np.asarray(routed_down, np.float32)
    gw_s = [_shuffle_gateup(routed_gate[e]) for e in range(E_ROUTED)]
    uw_s = [_shuffle_gateup(routed_up[e]) for e in range(E_ROUTED)]
    dw_s = [_shuffle_down(routed_down[e]) for e in range(E_ROUTED)]
    sg_s = _shuffle_gateup(np.asarray(shared_gate, np.float32))
    su_s = _shuffle_gateup(np.asarray(shared_up, np.float32))
    sd_s = _shuffle_down(np.asarray(shared_down, np.float32))

    # shared-token filler: spare seg1 slots first, then every core's seg2
    shared_ptr = [0]

    def take_shared(n):
        a = shared_ptr[0]
        b = min(T, a + n)
        shared_ptr[0] = b
        return np.arange(a, b)

    in_maps = []
    slot_tok = np.full((NCORES, C), -1, np.int64)
    for c in range(NCORES):
        xgf = np.zeros((C, H), np.float32)
        xwf = np.zeros((C, H), np.float32)
        if c < len(pieces):
            e, toks = pieces[c]
            n = len(toks)
            tok_ids = np.array([t for t, _ in toks], np.int64)
            wts = np.array([wt for _, wt in toks], np.float32)
            xgf[:n] = x2[tok_ids]
            xwf[:n] = x2[tok_ids] * wts[:, None]
            slot_tok[c, :n] = tok_ids
            w1g, w1u, w1d = gw_s[e], uw_s[e], dw_s[e]
        else:
            tok_ids = take_shared(c1)
            n = len(tok_ids)
            xgf[:n] = x2[tok_ids]
            xwf[:n] = x2[tok_ids]
            slot_tok[c, :n] = tok_ids
            w1g, w1u, w1d = sg_s, su_s, sd_s
        tok2 = take_shared(c2)
        n2 = len(tok2)
        xgf[c1:c1 + n2] = x2[tok2]
        xwf[c1:c1 + n2] = x2[tok2]
        slot_tok[c, c1:c1 + n2] = tok2
        in_maps.append({
            'xg': np.ascontiguousarray(xgf.T.astype(bf)),
            'xw': np.ascontiguousarray(xwf.T.astype(bf)),
            'g1': w1g, 'u1': w1u, 'd1': w1d,
            'g2': sg_s, 'u2': su_s, 'd2': sd_s,
        })
    assert shared_ptr[0] >= T, 'shared filler did not cover all tokens'

    key = (c1, c2)
    nc = _PROG_CACHE.get(key)
    if nc is None:
        nc = _build_program(key)
        _PROG_CACHE[key] = nc

    res = run_bass_kernel_spmd(nc, in_maps, list(range(NCORES)))
    LAST_RESULT = res

    # host combine: each token's 3 slots (1 shared + 2 routed) summed
    yt_flat = np.concatenate(
        [np.asarray(res.results[c]['yt'], np.float32).T for c in range(NCORES)],
        axis=0)                                            # [8*C, H]
    flat_tok = slot_tok.reshape(-1)
    valid = np.flatnonzero(flat_tok >= 0)
    order = valid[np.argsort(flat_tok[valid], kind='stable')]
    idx_mat = order.reshape(T, TOP_K + 1)                  # 3 slots per token
    out2 = yt_flat[idx_mat[:, 0]] + yt_flat[idx_mat[:, 1]] + yt_flat[idx_mat[:, 2]]
    return out2.reshape(B, S, H).astype(np.float32)


# revision 11
# speedup vs baseline: 1.0094x; 1.0094x over previous
"""DeepSeekMoE (B=2,S=2048,H=1024,I=2816, 7 routed experts top-2 + 1 shared) on 8 trn2 NeuronCores.

Strategy: collective-free unified expert-parallel.
  The shared expert has the same architecture as the routed experts, so every
  unit of work is "one MLP applied to one column" — a column is either a
  (token, routed-expert) slot or a (token, shared) slot.  12288 slot-columns
  total are packed into 8 cores x 2 uniform segments:
    seg1 (cap c1 = max expert load): core c < 7 carries routed expert c's
      entire token list; the remaining seg1 slots are shared-token filler.
    seg2 (cap c2): shared-token filler on every core.
  The top-2 combine weight is folded into the up-projection input on the host
  (xw = w * x), so slot outputs need no on-chip scaling, no scatter and no
  ReduceScatter: each core returns yt = down(silu(xg@G) * (xw@U)) [H, C] and
  the host sums each token's 3 slots (shared + 2 routed) — O(T*H) adds,
  ~0.01% of the FLOPs, same spirit as the host router.

  Device schedule per core (all matmuls bf16, f32 psum):
    1. gate/up: 22 I-chunks; per chunk stream 4 weight tiles (g/u x 2 segs)
       and run K=8-deep matmul groups over ~512-col tiles; silu on the scalar
       engine; h = silu(g)*u written to HT (bf16) by the vector engine.
    2. down: 8 H-chunks; per chunk stream 2 down-weight tiles (2 segs),
       K=22-deep matmul groups over the same column tiles, psum copied out on
       the scalar engine and DMAed to yt [H, C] f32.
  No collectives -> DMA streams freely; weights stream (2 expert sets/core,
  ~35 MB) far below the ~120 GB/s needed to keep pace with the PE.
"""

import math
import os
import sys
import types

import numpy as np
import ml_dtypes

for _p in ('/opt/trn_rl_repo', '/root/.axon_site/_ro/trn_rl_repo'):
    if os.path.isdir(_p) and _p not in sys.path:
        sys.path.append(_p)


def _install_profile_glue():
    """Optional: register the NTFF profile hook so trace=True/BASS_TRACE works
    under axon (the image's antenv lacks axon_hooks). Harmless if unavailable."""
    try:
        import antenv
        if 'antenv.axon_hooks' in sys.modules:
            return
        mod = types.ModuleType('antenv.axon_hooks')
        holder = [None]
        mod.set_axon_ntff_profile_hook = lambda h: holder.__setitem__(0, h)
        mod.get_axon_ntff_profile_hook = lambda: holder[0]
        sys.modules['antenv.axon_hooks'] = mod
        antenv.axon_hooks = mod
        so = '/opt/axon/libaxon_pjrt.so'
        if os.path.exists(so):
            from trn_agent_boot.trn_boot import _ntff_profile_via_ctypes
            hook = _ntff_profile_via_ctypes(so)
            if hook is not None:
                mod.set_axon_ntff_profile_hook(hook)
    except Exception:
        pass


_install_profile_glue()

import concourse.bass as bass
import concourse.mybir as mybir
from concourse.bass_utils import run_bass_kernel_spmd
from concourse.tile import TileContext

B, S, H, I = 2, 2048, 1024, 2816
E_ROUTED = 7
TOP_K = 2
T = B * S                  # 4096 tokens
NCORES = 8
KH = H // 128              # 8 contraction chunks over H (gate/up) = output chunks (down)
KI = I // 128              # 22 contraction chunks over I (down) = output chunks (gate/up)

F32 = mybir.dt.float32
BF16 = mybir.dt.bfloat16

LAST_RESULT = None         # BassKernelResults of the most recent run (for tests)

_PROG_CACHE = {}


def _split_sync_waits(nc, max_waits=1):
    """This container's walrus rejects >1 sync wait per instruction; spill
    extra waits onto same-engine NoOps placed just before the instruction."""
    for f in nc.m.functions:
        for bb in f.blocks:
            new_list = []
            changed = False
            for inst in bb.instructions:
                si = inst.sync_info
                if si is not None and si.on_wait is not None and len(si.on_wait) > max_waits:
                    waits = list(si.on_wait)
                    while len(waits) > max_waits:
                        chunk, waits = waits[:max_waits], waits[max_waits:]
                        nop = mybir.InstNoOp(
                            name=nc.get_next_instruction_name(),
                            engine=inst.engine, bass_nofuse=True,
                            sync_info=mybir.SyncInfo(on_wait=chunk, on_update=[]),
                        )
                        new_list.append(nop)
                    inst.sync_info = mybir.SyncInfo(
                        on_wait=waits, on_update=list(si.on_update or []))
                    changed = True
                new_list.append(inst)
            if changed:
                bb.instructions[:] = new_list


def _even_tiles(offset, total, width=512):
    """Split [offset, offset+total) into near-even tiles of <= width cols."""
    if total <= 0:
        return []
    n = (total + width - 1) // width
    base, rem = divmod(total, n)
    out = []
    c = offset
    for j in range(n):
        tn = base + (1 if j < rem else 0)
        out.append((c, tn))
        c += tn
    return out


def _build_program(caps):
    """Uniform SPMD program for segment capacities (c1, c2)."""
    c1, c2 = caps
    C = c1 + c2
    tiles1 = _even_tiles(0, c1)
    tiles2 = _even_tiles(c1, c2)

    nc = bass.Bass()
    xg = nc.declare_dram_parameter('xg', [H, C], BF16, isOutput=False)
    # xw only covers seg1: seg2 is always shared filler whose up-input is xg
    xw = nc.declare_dram_parameter('xw', [H, c1], BF16, isOutput=False)
    # gate/up weights arrive chunk-shuffled: [KI, 128, KH, 128] so each
    # per-I-chunk stream DMA reads 2KB-contiguous per partition.
    g1 = nc.declare_dram_parameter('g1', [KI, 128, KH, 128], BF16, isOutput=False)
    u1 = nc.declare_dram_parameter('u1', [KI, 128, KH, 128], BF16, isOutput=False)
    g2 = nc.declare_dram_parameter('g2', [KI, 128, KH, 128], BF16, isOutput=False)
    u2 = nc.declare_dram_parameter('u2', [KI, 128, KH, 128], BF16, isOutput=False)
    # down weights shuffled per output H-chunk: [KH, 128, KI, 128]
    d1 = nc.declare_dram_parameter('d1', [KH, 128, KI, 128], BF16, isOutput=False)
    d2 = nc.declare_dram_parameter('d2', [KH, 128, KI, 128], BF16, isOutput=False)
    yt = nc.declare_dram_parameter('yt', [H, C], F32, isOutput=True)

    xg_r = xg.rearrange('(k p) c -> p k c', p=128)
    xw_r = xw.rearrange('(k p) c -> p k c', p=128)
    warm_out = nc.dram_tensor('warm', [128, 128], F32)

    with TileContext(nc) as tc:
        with (
            tc.tile_pool(name='big', bufs=1) as bigp,
            tc.tile_pool(name='wstream', bufs=2) as wsp,
            tc.tile_pool(name='dstream', bufs=2) as dsp,
            tc.tile_pool(name='rtp', bufs=3) as rtp,
            tc.tile_pool(name='stg', bufs=3) as stg,
            tc.tile_pool(name='ps', bufs=8, space='PSUM') as psp,
        ):
            XG = bigp.tile([128, KH, C], BF16, tag='XG')
            XW = bigp.tile([128, KH, c1], BF16, tag='XW')
            HT = bigp.tile([128, KI, C], BF16, tag='HT')

            # ---- 0. PE clock warmup during input staging: the tensor engine
            # ramps to max frequency only after ~3us of continuous execution,
            # so burn dummy matmuls while the first DMAs land.
            wu = bigp.tile([128, 256], BF16, tag='wu')
            nc.vector.memset(wu[:, :], 0.0)
            wps = psp.tile([128, 512], F32, tag='ps', name='warm')
            WARM = 48
            for r in range(WARM):
                nc.tensor.matmul(wps[:, :256], lhsT=wu[:, :128],
                                 rhs=wu[:, :],
                                 start=(r == 0), stop=(r == WARM - 1))
            wst = stg.tile([128, 512], F32, tag='yst', name='warmst')
            nc.scalar.copy(out=wst[:, :128], in_=wps[:, :128])
            nc.sync.dma_start(out=warm_out[:, :], in_=wst[:, :128])

            # ---- 1. gate/up over 22 I-chunks
            for i in range(KI):
                gch1 = wsp.tile([128, KH, 128], BF16, tag='g1', name=f'g1_{i}')
                nc.sync.dma_start(out=gch1[:, :, :], in_=g1[i, :, :, :])
                gch2 = wsp.tile([128, KH, 128], BF16, tag='g2', name=f'g2_{i}')
                nc.sync.dma_start(out=gch2[:, :, :], in_=g2[i, :, :, :])
                uch1 = wsp.tile([128, KH, 128], BF16, tag='u1', name=f'u1_{i}')
                nc.sync.dma_start(out=uch1[:, :, :], in_=u1[i, :, :, :])
                uch2 = wsp.tile([128, KH, 128], BF16, tag='u2', name=f'u2_{i}')
                nc.sync.dma_start(out=uch2[:, :, :], in_=u2[i, :, :, :])
                if i == 0:
                    # stage x in consumption order (XG tile, then its XW pair)
                    # so the first groups start ~4us in and stay fed; staging
                    # is per-queue-bandwidth-bound, so order = readiness.
                    for (t0, tn) in tiles1:
                        for k in range(KH):
                            nc.sync.dma_start(
                                out=XG[:, k, t0:t0 + tn], in_=xg_r[:, k, t0:t0 + tn])
                        for k in range(KH):
                            nc.sync.dma_start(
                                out=XW[:, k, t0:t0 + tn], in_=xw_r[:, k, t0:t0 + tn])
                    for (t0, tn) in tiles2:
                        for k in range(KH):
                            nc.sync.dma_start(
                                out=XG[:, k, t0:t0 + tn], in_=xg_r[:, k, t0:t0 + tn])
                for (gch, uch, tiles, XUP, uoff) in (
                        (gch1, uch1, tiles1, XW, 0),
                        (gch2, uch2, tiles2, XG, None)):
                    for (t0, tn) in tiles:
                        gps = psp.tile([128, 512], F32, tag='ps', name=f'g{i}_{t0}')
                        for k in range(KH):
                            nc.tensor.matmul(
                                gps[:, :tn], lhsT=gch[:, k, :],
                                rhs=XG[:, k, t0:t0 + tn],
                                start=(k == 0), stop=(k == KH - 1))
                        at = rtp.tile([128, 512], F32, tag='at', name=f'at{i}_{t0}')
                        nc.scalar.activation(
                            out=at[:, :tn], in_=gps[:, :tn],
                            func=mybir.ActivationFunctionType.Silu)
                        ups = psp.tile([128, 512], F32, tag='ps', name=f'u{i}_{t0}')
                        for k in range(KH):
                            nc.tensor.matmul(
                                ups[:, :tn], lhsT=uch[:, k, :],
                                rhs=XUP[:, k, t0:t0 + tn],
                                start=(k == 0), stop=(k == KH - 1))
                        nc.vector.tensor_tensor(
                            out=HT[:, i, t0:t0 + tn],
                            in0=at[:, :tn], in1=ups[:, :tn],
                            op=mybir.AluOpType.mult)

            # ---- 2. down over 8 H-chunks
            for h in range(KH):
                dch1 = dsp.tile([128, KI, 128], BF16, tag='d1', name=f'd1_{h}')
                nc.sync.dma_start(out=dch1[:, :, :], in_=d1[h, :, :, :])
                dch2 = dsp.tile([128, KI, 128], BF16, tag='d2', name=f'd2_{h}')
                nc.sync.dma_start(out=dch2[:, :, :], in_=d2[h, :, :, :])
                for (dch, tiles) in ((dch1, tiles1), (dch2, tiles2)):
                    for (t0, tn) in tiles:
                        yps = psp.tile([128, 512], F32, tag='ps', name=f'y{h}_{t0}')
                        for k in range(KI):
                            nc.tensor.matmul(
                                yps[:, :tn], lhsT=dch[:, k, :],
                                rhs=HT[:, k, t0:t0 + tn],
                                start=(k == 0), stop=(k == KI - 1))
                        yst = stg.tile([128, 512], F32, tag='yst', name=f'ys{h}_{t0}')
                        nc.scalar.copy(out=yst[:, :tn], in_=yps[:, :tn])
                        nc.sync.dma_start(
                            out=yt[h * 128:(h + 1) * 128, t0:t0 + tn],
                            in_=yst[:, :tn])

    _split_sync_waits(nc)
    return nc


def _dispatch(x2, router_w, routing_bias):
    """Host router. Returns per-expert token lists [(token, weight)...]."""
    logits = x2 @ router_w + routing_bias            # [T, 7] fp32
    order = np.argsort(-logits, axis=1, kind='stable')[:, :TOP_K]
    probs = 1.0 / (1.0 + np.exp(-logits))
    rows = np.arange(T)
    s = probs[rows[:, None], order]                  # [T, 2]
    w = s / s.sum(axis=1, keepdims=True)             # renormalized combine weights

    lists = [[] for _ in range(E_ROUTED)]
    for k in range(TOP_K):
        for t, e, wt in zip(rows, order[:, k], w[:, k]):
            lists[e].append((int(t), float(wt)))
    return lists


def _shuffle_gateup(wmat):
    """[H, I] -> [KI, 128(H-part), KH, 128(I-cols)] bf16."""
    return np.ascontiguousarray(
        wmat.reshape(KH, 128, KI, 128).transpose(2, 1, 0, 3).astype(ml_dtypes.bfloat16))


def _shuffle_down(wmat):
    """[I, H] -> [KH(h), 128(I-part), KI(k), 128(H-cols)] bf16."""
    return np.ascontiguousarray(
        wmat.reshape(KI, 128, KH, 128).transpose(2, 1, 0, 3).astype(ml_dtypes.bfloat16))


def kernel(x, router_w, routing_bias, shared_gate, shared_up, shared_down,
           routed_gate, routed_up, routed_down):
    global LAST_RESULT
    x = np.asarray(x, np.float32)
    x2 = x.reshape(T, H)

    lists = _dispatch(x2, np.asarray(router_w, np.float32),
                      np.asarray(routing_bias, np.float32))

    # pieces: split any oversized expert so every piece fits one seg1 slot
    pieces = []                       # (expert_id, [(token, weight)...])
    for e in range(E_ROUTED):
        le = lists[e]
        nsplit = max(1, (len(le) + 2047) // 2048)
        step = (len(le) + nsplit - 1) // nsplit
        for a in range(0, len(le), step):
            pieces.append((e, le[a:a + step]))
    assert len(pieces) <= NCORES, 'expert pieces exceed core count'
    c1 = max(128, max(len(toks) for _, toks in pieces))
    n_spare = NCORES - len(pieces)
    c2 = max(0, -(-(T - n_spare * c1) // NCORES))
    c2 = max(c2, 1)
    C = c1 + c2

    bf = ml_dtypes.bfloat16
    routed_gate = np.asarray(routed_gate, np.float32)
    routed_up = np.asarray(routed_up, np.float32)
    routed_down = np.asarray(routed_down, np.float32)
    gw_s = [_shuffle_gateup(routed_gate[e]) for e in range(E_ROUTED)]
    uw_s = [_shuffle_gateup(routed_up[e]) for e in range(E_ROUTED)]
    dw_s = [_shuffle_down(routed_down[e]) for e in range(E_ROUTED)]
    sg_s = _shuffle_gateup(np.asarray(shared_gate, np.float32))
    su_s = _shuffle_gateup(np.asarray(shared_up, np.float32))
    sd_s = _shuffle_down(np.asarray(shared_down, np.float32))

    # shared-token filler: spare seg1 slots first, then every core's seg2
    shared_ptr = [0]

    def take_shared(n):
        a = shared_ptr[0]
        b = min(T, a + n)
        shared_ptr[0] = b
        return np.arange(a, b)

    in_maps = []
    slot_tok = np.full((NCORES, C), -1, np.int64)
    for c in range(NCORES):
        xgf = np.zeros((C, H), np.float32)
        xwf = np.zeros((c1, H), np.float32)
        if c < len(pieces):
            e, toks = pieces[c]
            n = len(toks)
            tok_ids = np.array([t for t, _ in toks], np.int64)
            wts = np.array([wt for _, wt in toks], np.float32)
            xgf[:n] = x2[tok_ids]
            xwf[:n] = x2[tok_ids] * wts[:, None]
            slot_tok[c, :n] = tok_ids
            w1g, w1u, w1d = gw_s[e], uw_s[e], dw_s[e]
        else:
            tok_ids = take_shared(c1)
            n = len(tok_ids)
            xgf[:n] = x2[tok_ids]
            xwf[:n] = x2[tok_ids]
            slot_tok[c, :n] = tok_ids
            w1g, w1u, w1d = sg_s, su_s, sd_s
        tok2 = take_shared(c2)
        n2 = len(tok2)
        xgf[c1:c1 + n2] = x2[tok2]
        slot_tok[c, c1:c1 + n2] = tok2
        in_maps.append({
            'xg': np.ascontiguousarray(xgf.T.astype(bf)),
            'xw': np.ascontiguousarray(xwf.T.astype(bf)),
            'g1': w1g, 'u1': w1u, 'd1': w1d,
            'g2': sg_s, 'u2': su_s, 'd2': sd_s,
        })
    assert shared_ptr[0] >= T, 'shared filler did not cover all tokens'

    key = (c1, c2)
    nc = _PROG_CACHE.get(key)
    if nc is None:
        nc = _build_program(key)
        _PROG_CACHE[key] = nc

    res = run_bass_kernel_spmd(nc, in_maps, list(range(NCORES)))
    LAST_RESULT = res

    # host combine: each token's 3 slots (1 shared + 2 routed) summed
    yt_flat = np.concatenate(
        [np.asarray(res.results[c]['yt'], np.float32).T for c in range(NCORES)],
        axis=0)                                            # [8*C, H]
    flat_tok = slot_tok.reshape(-1)
    valid = np.flatnonzero(flat_tok >= 0)
    order = valid[np.argsort(flat_tok[valid], kind='stable')]
    idx_mat = order.reshape(T, TOP_K + 1)                  # 3 slots per token
    out2 = yt_flat[idx_mat[:, 0]] + yt_flat[idx_mat[:, 1]] + yt_flat[idx_mat[:, 2]]
    return out2.reshape(B, S, H).astype(np.float32)


# revision 13
# speedup vs baseline: 1.0095x; 1.0002x over previous
"""DeepSeekMoE (B=2,S=2048,H=1024,I=2816, 7 routed experts top-2 + 1 shared) on 8 trn2 NeuronCores.

Strategy: collective-free unified expert-parallel.
  The shared expert has the same architecture as the routed experts, so every
  unit of work is "one MLP applied to one column" — a column is either a
  (token, routed-expert) slot or a (token, shared) slot.  12288 slot-columns
  total are packed into 8 cores x 2 uniform segments:
    seg1 (cap c1 = max expert load): core c < 7 carries routed expert c's
      entire token list; the remaining seg1 slots are shared-token filler.
    seg2 (cap c2): shared-token filler on every core.
  The top-2 combine weight is folded into the up-projection input on the host
  (xw = w * x), so slot outputs need no on-chip scaling, no scatter and no
  ReduceScatter: each core returns yt = down(silu(xg@G) * (xw@U)) [H, C] and
  the host sums each token's 3 slots (shared + 2 routed) — O(T*H) adds,
  ~0.01% of the FLOPs, same spirit as the host router.

  Device schedule per core (all matmuls bf16, f32 psum):
    1. gate/up: 22 I-chunks; per chunk stream 4 weight tiles (g/u x 2 segs)
       and run K=8-deep matmul groups over ~512-col tiles; silu on the scalar
       engine; h = silu(g)*u written to HT (bf16) by the vector engine.
    2. down: 8 H-chunks; per chunk stream 2 down-weight tiles (2 segs),
       K=22-deep matmul groups over the same column tiles, psum copied out on
       the scalar engine and DMAed to yt [H, C] f32.
  No collectives -> DMA streams freely; weights stream (2 expert sets/core,
  ~35 MB) far below the ~120 GB/s needed to keep pace with the PE.
"""

import math
import os
import sys
import types

import numpy as np
import ml_dtypes

for _p in ('/opt/trn_rl_repo', '/root/.axon_site/_ro/trn_rl_repo'):
    if os.path.isdir(_p) and _p not in sys.path:
        sys.path.append(_p)


def _install_profile_glue():
    """Optional: register the NTFF profile hook so trace=True/BASS_TRACE works
    under axon (the image's antenv lacks axon_hooks). Harmless if unavailable."""
    try:
        import antenv
        if 'antenv.axon_hooks' in sys.modules:
            return
        mod = types.ModuleType('antenv.axon_hooks')
        holder = [None]
        mod.set_axon_ntff_profile_hook = lambda h: holder.__setitem__(0, h)
        mod.get_axon_ntff_profile_hook = lambda: holder[0]
        sys.modules['antenv.axon_hooks'] = mod
        antenv.axon_hooks = mod
        so = '/opt/axon/libaxon_pjrt.so'
        if os.path.exists(so):
            from trn_agent_boot.trn_boot import _ntff_profile_via_ctypes
            hook = _ntff_profile_via_ctypes(so)
            if hook is not None:
                mod.set_axon_ntff_profile_hook(hook)
    except Exception:
        pass


_install_profile_glue()

import concourse.bass as bass
import concourse.mybir as mybir
from concourse.bass_utils import run_bass_kernel_spmd
from concourse.tile import TileContext

B, S, H, I = 2, 2048, 1024, 2816
E_ROUTED = 7
TOP_K = 2
T = B * S                  # 4096 tokens
NCORES = 8
KH = H // 128              # 8 contraction chunks over H (gate/up) = output chunks (down)
KI = I // 128              # 22 contraction chunks over I (down) = output chunks (gate/up)

F32 = mybir.dt.float32
BF16 = mybir.dt.bfloat16

LAST_RESULT = None         # BassKernelResults of the most recent run (for tests)

_PROG_CACHE = {}


def _split_sync_waits(nc, max_waits=1):
    """This container's walrus rejects >1 sync wait per instruction; spill
    extra waits onto same-engine NoOps placed just before the instruction."""
    for f in nc.m.functions:
        for bb in f.blocks:
            new_list = []
            changed = False
            for inst in bb.instructions:
                si = inst.sync_info
                if si is not None and si.on_wait is not None and len(si.on_wait) > max_waits:
                    waits = list(si.on_wait)
                    while len(waits) > max_waits:
                        chunk, waits = waits[:max_waits], waits[max_waits:]
                        nop = mybir.InstNoOp(
                            name=nc.get_next_instruction_name(),
                            engine=inst.engine, bass_nofuse=True,
                            sync_info=mybir.SyncInfo(on_wait=chunk, on_update=[]),
                        )
                        new_list.append(nop)
                    inst.sync_info = mybir.SyncInfo(
                        on_wait=waits, on_update=list(si.on_update or []))
                    changed = True
                new_list.append(inst)
            if changed:
                bb.instructions[:] = new_list


def _even_tiles(offset, total, width=512):
    """Split [offset, offset+total) into near-even tiles of <= width cols."""
    if total <= 0:
        return []
    n = (total + width - 1) // width
    base, rem = divmod(total, n)
    out = []
    c = offset
    for j in range(n):
        tn = base + (1 if j < rem else 0)
        out.append((c, tn))
        c += tn
    return out


def _build_program(caps):
    """Uniform SPMD program for segment capacities (c1, c2)."""
    c1, c2 = caps
    C = c1 + c2
    tiles1 = _even_tiles(0, c1)
    tiles2 = _even_tiles(c1, c2)

    nc = bass.Bass()
    xg = nc.declare_dram_parameter('xg', [H, C], BF16, isOutput=False)
    # xw only covers seg1: seg2 is always shared filler whose up-input is xg
    xw = nc.declare_dram_parameter('xw', [H, c1], BF16, isOutput=False)
    # gate/up weights arrive chunk-shuffled: [KI, 128, KH, 128] so each
    # per-I-chunk stream DMA reads 2KB-contiguous per partition.
    g1 = nc.declare_dram_parameter('g1', [KI, 128, KH, 128], BF16, isOutput=False)
    u1 = nc.declare_dram_parameter('u1', [KI, 128, KH, 128], BF16, isOutput=False)
    g2 = nc.declare_dram_parameter('g2', [KI, 128, KH, 128], BF16, isOutput=False)
    u2 = nc.declare_dram_parameter('u2', [KI, 128, KH, 128], BF16, isOutput=False)
    # down weights shuffled per output H-chunk: [KH, 128, KI, 128]
    d1 = nc.declare_dram_parameter('d1', [KH, 128, KI, 128], BF16, isOutput=False)
    d2 = nc.declare_dram_parameter('d2', [KH, 128, KI, 128], BF16, isOutput=False)
    yt = nc.declare_dram_parameter('yt', [H, C], F32, isOutput=True)

    xg_r = xg.rearrange('(k p) c -> p k c', p=128)
    xw_r = xw.rearrange('(k p) c -> p k c', p=128)
    warm_out = nc.dram_tensor('warm', [128, 128], F32)

    with TileContext(nc) as tc:
        with (
            tc.tile_pool(name='big', bufs=1) as bigp,
            tc.tile_pool(name='wstream', bufs=3) as wsp,
            tc.tile_pool(name='dstream', bufs=3) as dsp,
            tc.tile_pool(name='rtp', bufs=3) as rtp,
            tc.tile_pool(name='stg', bufs=3) as stg,
            tc.tile_pool(name='ps', bufs=8, space='PSUM') as psp,
        ):
            XG = bigp.tile([128, KH, C], BF16, tag='XG')
            XW = bigp.tile([128, KH, c1], BF16, tag='XW')
            HT = bigp.tile([128, KI, C], BF16, tag='HT')

            # ---- 0. PE clock warmup during input staging: the tensor engine
            # ramps to max frequency only after ~3us of continuous execution,
            # so burn dummy matmuls while the first DMAs land.
            wu = bigp.tile([128, 256], BF16, tag='wu')
            nc.vector.memset(wu[:, :], 0.0)
            wps = psp.tile([128, 512], F32, tag='ps', name='warm')
            WARM = 48
            for r in range(WARM):
                nc.tensor.matmul(wps[:, :256], lhsT=wu[:, :128],
                                 rhs=wu[:, :],
                                 start=(r == 0), stop=(r == WARM - 1))
            wst = stg.tile([128, 512], F32, tag='yst', name='warmst')
            nc.scalar.copy(out=wst[:, :128], in_=wps[:, :128])
            nc.sync.dma_start(out=warm_out[:, :], in_=wst[:, :128])

            # ---- 1. gate/up over 22 I-chunks
            for i in range(KI):
                gch1 = wsp.tile([128, KH, 128], BF16, tag='g1', name=f'g1_{i}')
                nc.sync.dma_start(out=gch1[:, :, :], in_=g1[i, :, :, :])
                gch2 = wsp.tile([128, KH, 128], BF16, tag='g2', name=f'g2_{i}')
                nc.sync.dma_start(out=gch2[:, :, :], in_=g2[i, :, :, :])
                if i == 0:
                    # stage x in consumption order (XG tile, then its XW pair)
                    # so the first groups start early and stay fed; staging
                    # is per-queue-bandwidth-bound, so order = readiness.
                    for (t0, tn) in tiles1[:1]:
                        for k in range(KH):
                            nc.sync.dma_start(
                                out=XG[:, k, t0:t0 + tn], in_=xg_r[:, k, t0:t0 + tn])
                        for k in range(KH):
                            nc.sync.dma_start(
                                out=XW[:, k, t0:t0 + tn], in_=xw_r[:, k, t0:t0 + tn])
                uch1 = wsp.tile([128, KH, 128], BF16, tag='u1', name=f'u1_{i}')
                nc.sync.dma_start(out=uch1[:, :, :], in_=u1[i, :, :, :])
                uch2 = wsp.tile([128, KH, 128], BF16, tag='u2', name=f'u2_{i}')
                nc.sync.dma_start(out=uch2[:, :, :], in_=u2[i, :, :, :])
                if i == 0:
                    for (t0, tn) in tiles1[1:]:
                        for k in range(KH):
                            nc.sync.dma_start(
                                out=XG[:, k, t0:t0 + tn], in_=xg_r[:, k, t0:t0 + tn])
                        for k in range(KH):
                            nc.sync.dma_start(
                                out=XW[:, k, t0:t0 + tn], in_=xw_r[:, k, t0:t0 + tn])
                    for (t0, tn) in tiles2:
                        for k in range(KH):
                            nc.sync.dma_start(
                                out=XG[:, k, t0:t0 + tn], in_=xg_r[:, k, t0:t0 + tn])
                for (gch, uch, tiles, XUP, uoff) in (
                        (gch1, uch1, tiles1, XW, 0),
                        (gch2, uch2, tiles2, XG, None)):
                    for (t0, tn) in tiles:
                        gps = psp.tile([128, 512], F32, tag='ps', name=f'g{i}_{t0}')
                        for k in range(KH):
                            nc.tensor.matmul(
                                gps[:, :tn], lhsT=gch[:, k, :],
                                rhs=XG[:, k, t0:t0 + tn],
                                start=(k == 0), stop=(k == KH - 1))
                        at = rtp.tile([128, 512], F32, tag='at', name=f'at{i}_{t0}')
                        nc.scalar.activation(
                            out=at[:, :tn], in_=gps[:, :tn],
                            func=mybir.ActivationFunctionType.Silu)
                        ups = psp.tile([128, 512], F32, tag='ps', name=f'u{i}_{t0}')
                        for k in range(KH):
                            nc.tensor.matmul(
                                ups[:, :tn], lhsT=uch[:, k, :],
                                rhs=XUP[:, k, t0:t0 + tn],
                                start=(k == 0), stop=(k == KH - 1))
                        nc.vector.tensor_tensor(
                            out=HT[:, i, t0:t0 + tn],
                            in0=at[:, :tn], in1=ups[:, :tn],
                            op=mybir.AluOpType.mult)

            # ---- 2. down over 8 H-chunks
            for h in range(KH):
                dch1 = dsp.tile([128, KI, 128], BF16, tag='d1', name=f'd1_{h}')
                nc.sync.dma_start(out=dch1[:, :, :], in_=d1[h, :, :, :])
                dch2 = dsp.tile([128, KI, 128], BF16, tag='d2', name=f'd2_{h}')
                nc.sync.dma_start(out=dch2[:, :, :], in_=d2[h, :, :, :])
                for (dch, tiles) in ((dch1, tiles1), (dch2, tiles2)):
                    for (t0, tn) in tiles:
                        yps = psp.tile([128, 512], F32, tag='ps', name=f'y{h}_{t0}')
                        for k in range(KI):
                            nc.tensor.matmul(
                                yps[:, :tn], lhsT=dch[:, k, :],
                                rhs=HT[:, k, t0:t0 + tn],
                                start=(k == 0), stop=(k == KI - 1))
                        yst = stg.tile([128, 512], F32, tag='yst', name=f'ys{h}_{t0}')
                        nc.scalar.copy(out=yst[:, :tn], in_=yps[:, :tn])
                        nc.sync.dma_start(
                            out=yt[h * 128:(h + 1) * 128, t0:t0 + tn],
                            in_=yst[:, :tn])

    _split_sync_waits(nc)
    return nc


def _dispatch(x2, router_w, routing_bias):
    """Host router. Returns per-expert token lists [(token, weight)...]."""
    logits = x2 @ router_w + routing_bias            # [T, 7] fp32
    order = np.argsort(-logits, axis=1, kind='stable')[:, :TOP_K]
    probs = 1.0 / (1.0 + np.exp(-logits))
    rows = np.arange(T)
    s = probs[rows[:, None], order]                  # [T, 2]
    w = s / s.sum(axis=1, keepdims=True)             # renormalized combine weights

    lists = [[] for _ in range(E_ROUTED)]
    for k in range(TOP_K):
        for t, e, wt in zip(rows, order[:, k], w[:, k]):
            lists[e].append((int(t), float(wt)))
    return lists


def _shuffle_gateup(wmat):
    """[H, I] -> [KI, 128(H-part), KH, 128(I-cols)] bf16."""
    return np.ascontiguousarray(
        wmat.reshape(KH, 128, KI, 128).transpose(2, 1, 0, 3).astype(ml_dtypes.bfloat16))


def _shuffle_down(wmat):
    """[I, H] -> [KH(h), 128(I-part), KI(k), 128(H-cols)] bf16."""
    return np.ascontiguousarray(
        wmat.reshape(KI, 128, KH, 128).transpose(2, 1, 0, 3).astype(ml_dtypes.bfloat16))


def kernel(x, router_w, routing_bias, shared_gate, shared_up, shared_down,
           routed_gate, routed_up, routed_down):
    global LAST_RESULT
    x = np.asarray(x, np.float32)
    x2 = x.reshape(T, H)

    lists = _dispatch(x2, np.asarray(router_w, np.float32),
                      np.asarray(routing_bias, np.float32))

    # pieces: split any oversized expert so every piece fits one seg1 slot
    pieces = []                       # (expert_id, [(token, weight)...])
    for e in range(E_ROUTED):
        le = lists[e]
        nsplit = max(1, (len(le) + 2047) // 2048)
        step = (len(le) + nsplit - 1) // nsplit
        for a in range(0, len(le), step):
            pieces.append((e, le[a:a + step]))
    assert len(pieces) <= NCORES, 'expert pieces exceed core count'
    c1 = max(128, max(len(toks) for _, toks in pieces))
    n_spare = NCORES - len(pieces)
    c2 = max(0, -(-(T - n_spare * c1) // NCORES))
    c2 = max(c2, 1)
    C = c1 + c2

    bf = ml_dtypes.bfloat16
    routed_gate = np.asarray(routed_gate, np.float32)
    routed_up = np.asarray(routed_up, np.float32)
    routed_down = np.asarray(routed_down, np.float32)
    gw_s = [_shuffle_gateup(routed_gate[e]) for e in range(E_ROUTED)]
    uw_s = [_shuffle_gateup(routed_up[e]) for e in range(E_ROUTED)]
    dw_s = [_shuffle_down(routed_down[e]) for e in range(E_ROUTED)]
    sg_s = _shuffle_gateup(np.asarray(shared_gate, np.float32))
    su_s = _shuffle_gateup(np.asarray(shared_up, np.float32))
    sd_s = _shuffle_down(np.asarray(shared_down, np.float32))

    # shared-token filler: spare seg1 slots first, then every core's seg2
    shared_ptr = [0]

    def take_shared(n):
        a = shared_ptr[0]
        b = min(T, a + n)
        shared_ptr[0] = b
        return np.arange(a, b)

    in_maps = []
    slot_tok = np.full((NCORES, C), -1, np.int64)
    for c in range(NCORES):
        xgf = np.zeros((C, H), np.float32)
        xwf = np.zeros((c1, H), np.float32)
        if c < len(pieces):
            e, toks = pieces[c]
            n = len(toks)
            tok_ids = np.array([t for t, _ in toks], np.int64)
            wts = np.array([wt for _, wt in toks], np.float32)
            xgf[:n] = x2[tok_ids]
            xwf[:n] = x2[tok_ids] * wts[:, None]
            slot_tok[c, :n] = tok_ids
            w1g, w1u, w1d = gw_s[e], uw_s[e], dw_s[e]
        else:
            tok_ids = take_shared(c1)
            n = len(tok_ids)
            xgf[:n] = x2[tok_ids]
            xwf[:n] = x2[tok_ids]
            slot_tok[c, :n] = tok_ids
            w1g, w1u, w1d = sg_s, su_s, sd_s
        tok2 = take_shared(c2)
        n2 = len(tok2)
        xgf[c1:c1 + n2] = x2[tok2]
        slot_tok[c, c1:c1 + n2] = tok2
        in_maps.append({
            'xg': np.ascontiguousarray(xgf.T.astype(bf)),
            'xw': np.ascontiguousarray(xwf.T.astype(bf)),
            'g1': w1g, 'u1': w1u, 'd1': w1d,
            'g2': sg_s, 'u2': su_s, 'd2': sd_s,
        })
    assert shared_ptr[0] >= T, 'shared filler did not cover all tokens'

    key = (c1, c2)
    nc = _PROG_CACHE.get(key)
    if nc is None:
        nc = _build_program(key)
        _PROG_CACHE[key] = nc

    res = run_bass_kernel_spmd(nc, in_maps, list(range(NCORES)))
    LAST_RESULT = res

    # host combine: each token's 3 slots (1 shared + 2 routed) summed
    yt_flat = np.concatenate(
        [np.asarray(res.results[c]['yt'], np.float32).T for c in range(NCORES)],
        axis=0)                                            # [8*C, H]
    flat_tok = slot_tok.reshape(-1)
    valid = np.flatnonzero(flat_tok >= 0)
    order = valid[np.argsort(flat_tok[valid], kind='stable')]
    idx_mat = order.reshape(T, TOP_K + 1)                  # 3 slots per token
    out2 = yt_flat[idx_mat[:, 0]] + yt_flat[idx_mat[:, 1]] + yt_flat[idx_mat[:, 2]]
    return out2.reshape(B, S, H).astype(np.float32)


# revision 22
# speedup vs baseline: 1.0705x; 1.0604x over previous
"""DeepSeekMoE (B=2,S=2048,H=1024,I=2816, 7 routed experts top-2 + 1 shared) on 8 trn2 NeuronCores.

Strategy: collective-free unified expert-parallel.
  The shared expert has the same architecture as the routed experts, so every
  unit of work is "one MLP applied to one column" — a column is either a
  (token, routed-expert) slot or a (token, shared) slot.  12288 slot-columns
  total are packed into 8 cores x 2 uniform segments:
    seg1 (cap c1 = max expert load): core c < 7 carries routed expert c's
      entire token list; the remaining seg1 slots are shared-token filler.
    seg2 (cap c2): shared-token filler on every core.
  The top-2 combine weight is folded into the up-projection input on the host
  (xw = w * x), so slot outputs need no on-chip scaling, no scatter and no
  ReduceScatter: each core returns yt = down(silu(xg@G) * (xw@U)) [H, C] and
  the host sums each token's 3 slots (shared + 2 routed) — O(T*H) adds,
  ~0.01% of the FLOPs, same spirit as the host router.

  Device schedule per core (all matmuls bf16, f32 psum):
    1. gate/up: 22 I-chunks; per chunk stream 4 weight tiles (g/u x 2 segs)
       and run K=8-deep matmul groups over ~512-col tiles; silu on the scalar
       engine; h = silu(g)*u written to HT (bf16) by the vector engine.
    2. down: 8 H-chunks; per chunk stream 2 down-weight tiles (2 segs),
       K=22-deep matmul groups over the same column tiles, psum copied out on
       the scalar engine and DMAed to yt [H, C] f32.
  No collectives -> DMA streams freely; weights stream (2 expert sets/core,
  ~35 MB) far below the ~120 GB/s needed to keep pace with the PE.
"""

import math
import os
import sys
import types

import numpy as np
import ml_dtypes

for _p in ('/opt/trn_rl_repo', '/root/.axon_site/_ro/trn_rl_repo'):
    if os.path.isdir(_p) and _p not in sys.path:
        sys.path.append(_p)


def _install_profile_glue():
    """Optional: register the NTFF profile hook so trace=True/BASS_TRACE works
    under axon (the image's antenv lacks axon_hooks). Harmless if unavailable."""
    try:
        import antenv
        if 'antenv.axon_hooks' in sys.modules:
            return
        mod = types.ModuleType('antenv.axon_hooks')
        holder = [None]
        mod.set_axon_ntff_profile_hook = lambda h: holder.__setitem__(0, h)
        mod.get_axon_ntff_profile_hook = lambda: holder[0]
        sys.modules['antenv.axon_hooks'] = mod
        antenv.axon_hooks = mod
        so = '/opt/axon/libaxon_pjrt.so'
        if os.path.exists(so):
            from trn_agent_boot.trn_boot import _ntff_profile_via_ctypes
            hook = _ntff_profile_via_ctypes(so)
            if hook is not None:
                mod.set_axon_ntff_profile_hook(hook)
    except Exception:
        pass


_install_profile_glue()

import concourse.bass as bass
import concourse.mybir as mybir
from concourse.bass_utils import run_bass_kernel_spmd
from concourse.tile import TileContext

B, S, H, I = 2, 2048, 1024, 2816
E_ROUTED = 7
TOP_K = 2
T = B * S                  # 4096 tokens
NCORES = 8
KH = H // 128              # 8 contraction chunks over H (gate/up) = output chunks (down)
KI = I // 128              # 22 contraction chunks over I (down) = output chunks (gate/up)

F32 = mybir.dt.float32
BF16 = mybir.dt.bfloat16

LAST_RESULT = None         # BassKernelResults of the most recent run (for tests)

_PROG_CACHE = {}


def _split_sync_waits(nc, max_waits=1):
    """This container's walrus rejects >1 sync wait per instruction; spill
    extra waits onto same-engine NoOps placed just before the instruction."""
    for f in nc.m.functions:
        for bb in f.blocks:
            new_list = []
            changed = False
            for inst in bb.instructions:
                si = inst.sync_info
                if si is not None and si.on_wait is not None and len(si.on_wait) > max_waits:
                    waits = list(si.on_wait)
                    while len(waits) > max_waits:
                        chunk, waits = waits[:max_waits], waits[max_waits:]
                        nop = mybir.InstNoOp(
                            name=nc.get_next_instruction_name(),
                            engine=inst.engine, bass_nofuse=True,
                            sync_info=mybir.SyncInfo(on_wait=chunk, on_update=[]),
                        )
                        new_list.append(nop)
                    inst.sync_info = mybir.SyncInfo(
                        on_wait=waits, on_update=list(si.on_update or []))
                    changed = True
                new_list.append(inst)
            if changed:
                bb.instructions[:] = new_list


def _even_tiles(offset, total, width=512):
    """Split [offset, offset+total) into near-even tiles of <= width cols."""
    if total <= 0:
        return []
    n = (total + width - 1) // width
    base, rem = divmod(total, n)
    out = []
    c = offset
    for j in range(n):
        tn = base + (1 if j < rem else 0)
        out.append((c, tn))
        c += tn
    return out


WARM_DEFAULT = int(os.environ.get('KWARM', '0'))
STAGE_MODE = os.environ.get('KSTAGE', 'coarse')
XSPLIT = int(os.environ.get('KXSPLIT', '1'))
I0_GATE_FIRST = os.environ.get('KI0', 'gf') == 'gf'


def _build_program(caps):
    """Uniform SPMD program for segment capacities (c1, c2)."""
    c1, c2 = caps
    C = c1 + c2
    tiles1 = _even_tiles(0, c1)
    tiles2 = _even_tiles(c1, c2)

    nc = bass.Bass()
    xg = nc.declare_dram_parameter('xg', [H, C], BF16, isOutput=False)
    # xw only covers seg1: seg2 is always shared filler whose up-input is xg
    xw = nc.declare_dram_parameter('xw', [H, c1], BF16, isOutput=False)
    # gate/up weights arrive chunk-shuffled: [KI, 128, KH, 128] so each
    # per-I-chunk stream DMA reads 2KB-contiguous per partition.
    g1 = nc.declare_dram_parameter('g1', [KI, 128, KH, 128], BF16, isOutput=False)
    u1 = nc.declare_dram_parameter('u1', [KI, 128, KH, 128], BF16, isOutput=False)
    g2 = nc.declare_dram_parameter('g2', [KI, 128, KH, 128], BF16, isOutput=False)
    u2 = nc.declare_dram_parameter('u2', [KI, 128, KH, 128], BF16, isOutput=False)
    # down weights shuffled per output H-chunk: [KH, 128, KI, 128]
    d1 = nc.declare_dram_parameter('d1', [KH, 128, KI, 128], BF16, isOutput=False)
    d2 = nc.declare_dram_parameter('d2', [KH, 128, KI, 128], BF16, isOutput=False)
    yt = nc.declare_dram_parameter('yt', [H, C], F32, isOutput=True)

    xg_r = xg.rearrange('(k p) c -> p k c', p=128)
    xw_r = xw.rearrange('(k p) c -> p k c', p=128)
    warm_out = nc.dram_tensor('warm', [128, 128], F32)

    with TileContext(nc) as tc:
        with (
            tc.tile_pool(name='big', bufs=1) as bigp,
            tc.tile_pool(name='wstream', bufs=3) as wsp,
            tc.tile_pool(name='dstream', bufs=3) as dsp,
            tc.tile_pool(name='rtp', bufs=5) as rtp,
            tc.tile_pool(name='stg', bufs=3) as stg,
            tc.tile_pool(name='ps', bufs=8, space='PSUM') as psp,
        ):
            XG = bigp.tile([128, KH, C], BF16, tag='XG')
            XW = bigp.tile([128, KH, c1], BF16, tag='XW')
            HT = bigp.tile([128, KI, C], BF16, tag='HT')

            # ---- 0. PE clock warmup during input staging: the tensor engine
            # ramps to max frequency only after ~3us of continuous execution,
            # so burn dummy matmuls while the first DMAs land.
            WARM = WARM_DEFAULT
            if WARM > 0:
                wu = bigp.tile([128, 256], BF16, tag='wu')
                nc.vector.memset(wu[:, :], 0.0)
                wps = psp.tile([128, 512], F32, tag='ps', name='warm')
                for r in range(WARM):
                    nc.tensor.matmul(wps[:, :256], lhsT=wu[:, :128],
                                     rhs=wu[:, :],
                                     start=(r == 0), stop=(r == WARM - 1))
                wst = stg.tile([128, 512], F32, tag='yst', name='warmst')
                nc.scalar.copy(out=wst[:, :128], in_=wps[:, :128])
                nc.sync.dma_start(out=warm_out[:, :], in_=wst[:, :128])

            # ---- 1. gate/up over 22 I-chunks
            for i in range(KI):
                gch1 = wsp.tile([128, KH, 128], BF16, tag='g1', name=f'g1_{i}')
                nc.sync.dma_start(out=gch1[:, :, :], in_=g1[i, :, :, :])
                gch2 = wsp.tile([128, KH, 128], BF16, tag='g2', name=f'g2_{i}')
                nc.sync.dma_start(out=gch2[:, :, :], in_=g2[i, :, :, :])
                if i == 0 and STAGE_MODE == 'pair':
                    # stage x in consumption order (XG tile, then its XW pair)
                    # so the first groups start early and stay fed; staging
                    # is per-queue-bandwidth-bound, so order = readiness.
                    for (t0, tn) in tiles1[:1]:
                        for k in range(KH):
                            nc.sync.dma_start(
                                out=XG[:, k, t0:t0 + tn], in_=xg_r[:, k, t0:t0 + tn])
                        for k in range(KH):
                            nc.sync.dma_start(
                                out=XW[:, k, t0:t0 + tn], in_=xw_r[:, k, t0:t0 + tn])
                uch1 = wsp.tile([128, KH, 128], BF16, tag='u1', name=f'u1_{i}')
                nc.sync.dma_start(out=uch1[:, :, :], in_=u1[i, :, :, :])
                uch2 = wsp.tile([128, KH, 128], BF16, tag='u2', name=f'u2_{i}')
                nc.sync.dma_start(out=uch2[:, :, :], in_=u2[i, :, :, :])
                if i == 0 and STAGE_MODE == 'pair':
                    for (t0, tn) in tiles1[1:]:
                        for k in range(KH):
                            nc.sync.dma_start(
                                out=XG[:, k, t0:t0 + tn], in_=xg_r[:, k, t0:t0 + tn])
                        for k in range(KH):
                            nc.sync.dma_start(
                                out=XW[:, k, t0:t0 + tn], in_=xw_r[:, k, t0:t0 + tn])
                    for (t0, tn) in tiles2:
                        for k in range(KH):
                            nc.sync.dma_start(
                                out=XG[:, k, t0:t0 + tn], in_=xg_r[:, k, t0:t0 + tn])
                if i == 0 and STAGE_MODE == 'coarse':
                    for k in range(KH):
                        for (q0, qn) in _even_tiles(0, C, -(-C // XSPLIT)):
                            nc.sync.dma_start(out=XG[:, k, q0:q0 + qn],
                                              in_=xg_r[:, k, q0:q0 + qn])
                    for k in range(KH):
                        for (q0, qn) in _even_tiles(0, c1, -(-c1 // XSPLIT)):
                            nc.sync.dma_start(out=XW[:, k, q0:q0 + qn],
                                              in_=xw_r[:, k, q0:q0 + qn])
                work = [(gch1, uch1, t0, tn, XW) for (t0, tn) in tiles1] + \
                       [(gch2, uch2, t0, tn, XG) for (t0, tn) in tiles2]
                gate_first = I0_GATE_FIRST and i == 0
                ats = {}

                def gate_part(gch, t0, tn):
                    gps = psp.tile([128, 512], F32, tag='ps', name=f'g{i}_{t0}')
                    for k in range(KH):
                        nc.tensor.matmul(
                            gps[:, :tn], lhsT=gch[:, k, :],
                            rhs=XG[:, k, t0:t0 + tn],
                            start=(k == 0), stop=(k == KH - 1))
                    at = rtp.tile([128, 512], F32, tag='at', name=f'at{i}_{t0}')
                    nc.scalar.activation(
                        out=at[:, :tn], in_=gps[:, :tn],
                        func=mybir.ActivationFunctionType.Silu)
                    ats[t0] = at

                def up_part(uch, t0, tn, XUP):
                    ups = psp.tile([128, 512], F32, tag='ps', name=f'u{i}_{t0}')
                    for k in range(KH):
                        nc.tensor.matmul(
                            ups[:, :tn], lhsT=uch[:, k, :],
                            rhs=XUP[:, k, t0:t0 + tn],
                            start=(k == 0), stop=(k == KH - 1))
                    nc.vector.tensor_tensor(
                        out=HT[:, i, t0:t0 + tn],
                        in0=ats[t0][:, :tn], in1=ups[:, :tn],
                        op=mybir.AluOpType.mult)

                if gate_first:
                    for (gch, uch, t0, tn, XUP) in work:
                        gate_part(gch, t0, tn)
                    for (gch, uch, t0, tn, XUP) in work:
                        up_part(uch, t0, tn, XUP)
                else:
                    for (gch, uch, t0, tn, XUP) in work:
                        gate_part(gch, t0, tn)
                        up_part(uch, t0, tn, XUP)

            # ---- 2. down over 8 H-chunks
            for h in range(KH):
                dch1 = dsp.tile([128, KI, 128], BF16, tag='d1', name=f'd1_{h}')
                nc.sync.dma_start(out=dch1[:, :, :], in_=d1[h, :, :, :])
                dch2 = dsp.tile([128, KI, 128], BF16, tag='d2', name=f'd2_{h}')
                nc.sync.dma_start(out=dch2[:, :, :], in_=d2[h, :, :, :])
                for (dch, tiles) in ((dch1, tiles1), (dch2, tiles2)):
                    for (t0, tn) in tiles:
                        yps = psp.tile([128, 512], F32, tag='ps', name=f'y{h}_{t0}')
                        for k in range(KI):
                            nc.tensor.matmul(
                                yps[:, :tn], lhsT=dch[:, k, :],
                                rhs=HT[:, k, t0:t0 + tn],
                                start=(k == 0), stop=(k == KI - 1))
                        yst = stg.tile([128, 512], F32, tag='yst', name=f'ys{h}_{t0}')
                        nc.scalar.copy(out=yst[:, :tn], in_=yps[:, :tn])
                        nc.sync.dma_start(
                            out=yt[h * 128:(h + 1) * 128, t0:t0 + tn],
                            in_=yst[:, :tn])

    _split_sync_waits(nc)
    return nc


def _dispatch(x2, router_w, routing_bias):
    """Host router. Returns per-expert token lists [(token, weight)...]."""
    logits = x2 @ router_w + routing_bias            # [T, 7] fp32
    order = np.argsort(-logits, axis=1, kind='stable')[:, :TOP_K]
    probs = 1.0 / (1.0 + np.exp(-logits))
    rows = np.arange(T)
    s = probs[rows[:, None], order]                  # [T, 2]
    w = s / s.sum(axis=1, keepdims=True)             # renormalized combine weights

    lists = [[] for _ in range(E_ROUTED)]
    for k in range(TOP_K):
        for t, e, wt in zip(rows, order[:, k], w[:, k]):
            lists[e].append((int(t), float(wt)))
    return lists


def _shuffle_gateup(wmat):
    """[H, I] -> [KI, 128(H-part), KH, 128(I-cols)] bf16."""
    return np.ascontiguousarray(
        wmat.reshape(KH, 128, KI, 128).transpose(2, 1, 0, 3).astype(ml_dtypes.bfloat16))


def _shuffle_down(wmat):
    """[I, H] -> [KH(h), 128(I-part), KI(k), 128(H-cols)] bf16."""
    return np.ascontiguousarray(
        wmat.reshape(KI, 128, KH, 128).transpose(2, 1, 0, 3).astype(ml_dtypes.bfloat16))


def kernel(x, router_w, routing_bias, shared_gate, shared_up, shared_down,
           routed_gate, routed_up, routed_down):
    global LAST_RESULT
    x = np.asarray(x, np.float32)
    x2 = x.reshape(T, H)

    lists = _dispatch(x2, np.asarray(router_w, np.float32),
                      np.asarray(routing_bias, np.float32))

    # pieces: split any oversized expert so every piece fits one seg1 slot
    pieces = []                       # (expert_id, [(token, weight)...])
    for e in range(E_ROUTED):
        le = lists[e]
        nsplit = max(1, (len(le) + 2047) // 2048)
        step = (len(le) + nsplit - 1) // nsplit
        for a in range(0, len(le), step):
            pieces.append((e, le[a:a + step]))
    assert len(pieces) <= NCORES, 'expert pieces exceed core count'
    c1 = max(128, max(len(toks) for _, toks in pieces))
    n_spare = NCORES - len(pieces)
    c2 = max(0, -(-(T - n_spare * c1) // NCORES))
    c2 = max(c2, 1)
    C = c1 + c2

    bf = ml_dtypes.bfloat16
    routed_gate = np.asarray(routed_gate, np.float32)
    routed_up = np.asarray(routed_up, np.float32)
    routed_down = np.asarray(routed_down, np.float32)
    gw_s = [_shuffle_gateup(routed_gate[e]) for e in range(E_ROUTED)]
    uw_s = [_shuffle_gateup(routed_up[e]) for e in range(E_ROUTED)]
    dw_s = [_shuffle_down(routed_down[e]) for e in range(E_ROUTED)]
    sg_s = _shuffle_gateup(np.asarray(shared_gate, np.float32))
    su_s = _shuffle_gateup(np.asarray(shared_up, np.float32))
    sd_s = _shuffle_down(np.asarray(shared_down, np.float32))

    # shared-token filler: spare seg1 slots first, then every core's seg2
    shared_ptr = [0]

    def take_shared(n):
        a = shared_ptr[0]
        b = min(T, a + n)
        shared_ptr[0] = b
        return np.arange(a, b)

    in_maps = []
    slot_tok = np.full((NCORES, C), -1, np.int64)
    for c in range(NCORES):
        xgf = np.zeros((C, H), np.float32)
        xwf = np.zeros((c1, H), np.float32)
        if c < len(pieces):
            e, toks = pieces[c]
            n = len(toks)
            tok_ids = np.array([t for t, _ in toks], np.int64)
            wts = np.array([wt for _, wt in toks], np.float32)
            xgf[:n] = x2[tok_ids]
            xwf[:n] = x2[tok_ids] * wts[:, None]
            slot_tok[c, :n] = tok_ids
            w1g, w1u, w1d = gw_s[e], uw_s[e], dw_s[e]
        else:
            tok_ids = take_shared(c1)
            n = len(tok_ids)
            xgf[:n] = x2[tok_ids]
            xwf[:n] = x2[tok_ids]
            slot_tok[c, :n] = tok_ids
            w1g, w1u, w1d = sg_s, su_s, sd_s
        tok2 = take_shared(c2)
        n2 = len(tok2)
        xgf[c1:c1 + n2] = x2[tok2]
        slot_tok[c, c1:c1 + n2] = tok2
        in_maps.append({
            'xg': np.ascontiguousarray(xgf.T.astype(bf)),
            'xw': np.ascontiguousarray(xwf.T.astype(bf)),
            'g1': w1g, 'u1': w1u, 'd1': w1d,
            'g2': sg_s, 'u2': su_s, 'd2': sd_s,
        })
    assert shared_ptr[0] >= T, 'shared filler did not cover all tokens'

    key = (c1, c2)
    nc = _PROG_CACHE.get(key)
    if nc is None:
        nc = _build_program(key)
        _PROG_CACHE[key] = nc

    res = run_bass_kernel_spmd(nc, in_maps, list(range(NCORES)))
    LAST_RESULT = res

    # host combine: each token's 3 slots (1 shared + 2 routed) summed
    yt_flat = np.concatenate(
        [np.asarray(res.results[c]['yt'], np.float32).T for c in range(NCORES)],
        axis=0)                                            # [8*C, H]
    flat_tok = slot_tok.reshape(-1)
    valid = np.flatnonzero(flat_tok >= 0)
    order = valid[np.argsort(flat_tok[valid], kind='stable')]
    idx_mat = order.reshape(T, TOP_K + 1)                  # 3 slots per token
    out2 = yt_flat[idx_mat[:, 0]] + yt_flat[idx_mat[:, 1]] + yt_flat[idx_mat[:, 2]]
    return out2.reshape(B, S, H).astype(np.float32)


# revision 31
# speedup vs baseline: 1.0772x; 1.0062x over previous
"""DeepSeekMoE (B=2,S=2048,H=1024,I=2816, 7 routed experts top-2 + 1 shared) on 8 trn2 NeuronCores.

Strategy: collective-free unified expert-parallel.
  The shared expert has the same architecture as the routed experts, so every
  unit of work is "one MLP applied to one column" — a column is either a
  (token, routed-expert) slot or a (token, shared) slot.  12288 slot-columns
  total are packed into 8 cores x 2 uniform segments:
    seg1 (cap c1 = max expert load): core c < 7 carries routed expert c's
      entire token list; the remaining seg1 slots are shared-token filler.
    seg2 (cap c2): shared-token filler on every core.
  Slot outputs need no on-chip scaling, no scatter and no ReduceScatter:
  each core returns raw yt = down(silu(xg@G) * (xg@U)) [H, C] and the host
  computes each token's weighted 3-slot sum (shared + top-2 routed, combine
  weights applied there) — O(T*H) flops, ~0.01% of the total, same spirit
  as the host router.

  Device schedule per core (all matmuls bf16, f32 psum):
    1. gate/up: 22 I-chunks; per chunk stream 4 weight tiles (g/u x 2 segs)
       and run K=8-deep matmul groups over ~512-col tiles; silu on the scalar
       engine; h = silu(g)*u written to HT (bf16) by the vector engine.
    2. down: 8 H-chunks; per chunk stream 2 down-weight tiles (2 segs),
       K=22-deep matmul groups over the same column tiles, psum copied out on
       the scalar engine and DMAed to yt [H, C] f32.
  No collectives -> DMA streams freely; weights stream (2 expert sets/core,
  ~35 MB) far below the ~120 GB/s needed to keep pace with the PE.
"""

import math
import os
import sys
import types

import numpy as np
import ml_dtypes

for _p in ('/opt/trn_rl_repo', '/root/.axon_site/_ro/trn_rl_repo'):
    if os.path.isdir(_p) and _p not in sys.path:
        sys.path.append(_p)


def _install_profile_glue():
    """Optional: register the NTFF profile hook so trace=True/BASS_TRACE works
    under axon (the image's antenv lacks axon_hooks). Harmless if unavailable."""
    try:
        import antenv
        if 'antenv.axon_hooks' in sys.modules:
            return
        mod = types.ModuleType('antenv.axon_hooks')
        holder = [None]
        mod.set_axon_ntff_profile_hook = lambda h: holder.__setitem__(0, h)
        mod.get_axon_ntff_profile_hook = lambda: holder[0]
        sys.modules['antenv.axon_hooks'] = mod
        antenv.axon_hooks = mod
        so = '/opt/axon/libaxon_pjrt.so'
        if os.path.exists(so):
            from trn_agent_boot.trn_boot import _ntff_profile_via_ctypes
            hook = _ntff_profile_via_ctypes(so)
            if hook is not None:
                mod.set_axon_ntff_profile_hook(hook)
    except Exception:
        pass


_install_profile_glue()

import concourse.bass as bass
import concourse.mybir as mybir
from concourse.bass_utils import run_bass_kernel_spmd
from concourse.tile import TileContext

B, S, H, I = 2, 2048, 1024, 2816
E_ROUTED = 7
TOP_K = 2
T = B * S                  # 4096 tokens
NCORES = 8
KH = H // 128              # 8 contraction chunks over H (gate/up) = output chunks (down)
KI = I // 128              # 22 contraction chunks over I (down) = output chunks (gate/up)

F32 = mybir.dt.float32
BF16 = mybir.dt.bfloat16

LAST_RESULT = None         # BassKernelResults of the most recent run (for tests)

_PROG_CACHE = {}


def _split_sync_waits(nc, max_waits=1):
    """This container's walrus rejects >1 sync wait per instruction; spill
    extra waits onto same-engine NoOps placed just before the instruction."""
    for f in nc.m.functions:
        for bb in f.blocks:
            new_list = []
            changed = False
            for inst in bb.instructions:
                si = inst.sync_info
                if si is not None and si.on_wait is not None and len(si.on_wait) > max_waits:
                    waits = list(si.on_wait)
                    while len(waits) > max_waits:
                        chunk, waits = waits[:max_waits], waits[max_waits:]
                        nop = mybir.InstNoOp(
                            name=nc.get_next_instruction_name(),
                            engine=inst.engine, bass_nofuse=True,
                            sync_info=mybir.SyncInfo(on_wait=chunk, on_update=[]),
                        )
                        new_list.append(nop)
                    inst.sync_info = mybir.SyncInfo(
                        on_wait=waits, on_update=list(si.on_update or []))
                    changed = True
                new_list.append(inst)
            if changed:
                bb.instructions[:] = new_list


def _even_tiles(offset, total, width=512):
    """Split [offset, offset+total) into near-even tiles of <= width cols."""
    if total <= 0:
        return []
    n = (total + width - 1) // width
    base, rem = divmod(total, n)
    out = []
    c = offset
    for j in range(n):
        tn = base + (1 if j < rem else 0)
        out.append((c, tn))
        c += tn
    return out


WARM_DEFAULT = int(os.environ.get('KWARM', '0'))
STAGE_MODE = os.environ.get('KSTAGE', 'coarse')
XSPLIT = int(os.environ.get('KXSPLIT', '1'))
I0_GATE_FIRST = os.environ.get('KI0', 'gf') == 'gf'


def _build_program(caps):
    """Uniform SPMD program for segment capacities (c1, c2)."""
    c1, c2 = caps
    C = c1 + c2
    tiles1 = _even_tiles(0, c1)
    tiles2 = _even_tiles(c1, c2)

    nc = bass.Bass()
    # one gathered x copy serves gate AND up: the top-2 combine weight is
    # applied by the host to the returned slot outputs, not on-chip.
    xg = nc.declare_dram_parameter('xg', [H, C], BF16, isOutput=False)
    # gate/up weights arrive chunk-shuffled: [KI, 128, KH, 128] so each
    # per-I-chunk stream DMA reads 2KB-contiguous per partition.
    g1 = nc.declare_dram_parameter('g1', [KI, 128, KH, 128], BF16, isOutput=False)
    u1 = nc.declare_dram_parameter('u1', [KI, 128, KH, 128], BF16, isOutput=False)
    g2 = nc.declare_dram_parameter('g2', [KI, 128, KH, 128], BF16, isOutput=False)
    u2 = nc.declare_dram_parameter('u2', [KI, 128, KH, 128], BF16, isOutput=False)
    # down weights shuffled per output H-chunk: [KH, 128, KI, 128]
    d1 = nc.declare_dram_parameter('d1', [KH, 128, KI, 128], BF16, isOutput=False)
    d2 = nc.declare_dram_parameter('d2', [KH, 128, KI, 128], BF16, isOutput=False)
    yt = nc.declare_dram_parameter('yt', [H, C], F32, isOutput=True)

    xg_r = xg.rearrange('(k p) c -> p k c', p=128)
    warm_out = nc.dram_tensor('warm', [128, 128], F32)

    with TileContext(nc) as tc:
        with (
            tc.tile_pool(name='big', bufs=1) as bigp,
            tc.tile_pool(name='wstream', bufs=3) as wsp,
            tc.tile_pool(name='dstream', bufs=3) as dsp,
            tc.tile_pool(name='rtp', bufs=5) as rtp,
            tc.tile_pool(name='stg', bufs=3) as stg,
            tc.tile_pool(name='ps', bufs=8, space='PSUM') as psp,
        ):
            XG = bigp.tile([128, KH, C], BF16, tag='XG')
            HT = bigp.tile([128, KI, C], BF16, tag='HT')

            # ---- 0. PE clock warmup during input staging: the tensor engine
            # ramps to max frequency only after ~3us of continuous execution,
            # so burn dummy matmuls while the first DMAs land.
            WARM = WARM_DEFAULT
            if WARM > 0:
                wu = bigp.tile([128, 256], BF16, tag='wu')
                nc.vector.memset(wu[:, :], 0.0)
                wps = psp.tile([128, 512], F32, tag='ps', name='warm')
                for r in range(WARM):
                    nc.tensor.matmul(wps[:, :256], lhsT=wu[:, :128],
                                     rhs=wu[:, :],
                                     start=(r == 0), stop=(r == WARM - 1))
                wst = stg.tile([128, 512], F32, tag='yst', name='warmst')
                nc.scalar.copy(out=wst[:, :128], in_=wps[:, :128])
                nc.sync.dma_start(out=warm_out[:, :], in_=wst[:, :128])

            # ---- 1. gate/up over 22 I-chunks
            for i in range(KI):
                gch1 = wsp.tile([128, KH, 128], BF16, tag='g1', name=f'g1_{i}')
                nc.sync.dma_start(out=gch1[:, :, :], in_=g1[i, :, :, :])
                gch2 = wsp.tile([128, KH, 128], BF16, tag='g2', name=f'g2_{i}')
                nc.sync.dma_start(out=gch2[:, :, :], in_=g2[i, :, :, :])
                uch1 = wsp.tile([128, KH, 128], BF16, tag='u1', name=f'u1_{i}')
                nc.sync.dma_start(out=uch1[:, :, :], in_=u1[i, :, :, :])
                uch2 = wsp.tile([128, KH, 128], BF16, tag='u2', name=f'u2_{i}')
                nc.sync.dma_start(out=uch2[:, :, :], in_=u2[i, :, :, :])
                if i == 0:
                    for k in range(KH):
                        for (q0, qn) in _even_tiles(0, C, -(-C // XSPLIT)):
                            nc.sync.dma_start(out=XG[:, k, q0:q0 + qn],
                                              in_=xg_r[:, k, q0:q0 + qn])
                work = [(gch1, uch1, t0, tn) for (t0, tn) in tiles1] + \
                       [(gch2, uch2, t0, tn) for (t0, tn) in tiles2]
                gate_first = I0_GATE_FIRST and i == 0
                ats = {}

                def gate_part(gch, t0, tn):
                    gps = psp.tile([128, 512], F32, tag='ps', name=f'g{i}_{t0}')
                    for k in range(KH):
                        nc.tensor.matmul(
                            gps[:, :tn], lhsT=gch[:, k, :],
                            rhs=XG[:, k, t0:t0 + tn],
                            start=(k == 0), stop=(k == KH - 1))
                    at = rtp.tile([128, 512], F32, tag='at', name=f'at{i}_{t0}')
                    nc.scalar.activation(
                        out=at[:, :tn], in_=gps[:, :tn],
                        func=mybir.ActivationFunctionType.Silu)
                    ats[t0] = at

                def up_part(uch, t0, tn):
                    ups = psp.tile([128, 512], F32, tag='ps', name=f'u{i}_{t0}')
                    for k in range(KH):
                        nc.tensor.matmul(
                            ups[:, :tn], lhsT=uch[:, k, :],
                            rhs=XG[:, k, t0:t0 + tn],
                            start=(k == 0), stop=(k == KH - 1))
                    nc.vector.tensor_tensor(
                        out=HT[:, i, t0:t0 + tn],
                        in0=ats[t0][:, :tn], in1=ups[:, :tn],
                        op=mybir.AluOpType.mult)

                if gate_first:
                    for (gch, uch, t0, tn) in work:
                        gate_part(gch, t0, tn)
                    for (gch, uch, t0, tn) in work:
                        up_part(uch, t0, tn)
                else:
                    for (gch, uch, t0, tn) in work:
                        gate_part(gch, t0, tn)
                        up_part(uch, t0, tn)

            # ---- 2. down over 8 H-chunks
            for h in range(KH):
                dch1 = dsp.tile([128, KI, 128], BF16, tag='d1', name=f'd1_{h}')
                nc.sync.dma_start(out=dch1[:, :, :], in_=d1[h, :, :, :])
                dch2 = dsp.tile([128, KI, 128], BF16, tag='d2', name=f'd2_{h}')
                nc.sync.dma_start(out=dch2[:, :, :], in_=d2[h, :, :, :])
                for (dch, tiles) in ((dch1, tiles1), (dch2, tiles2)):
                    for (t0, tn) in tiles:
                        yps = psp.tile([128, 512], F32, tag='ps', name=f'y{h}_{t0}')
                        for k in range(KI):
                            nc.tensor.matmul(
                                yps[:, :tn], lhsT=dch[:, k, :],
                                rhs=HT[:, k, t0:t0 + tn],
                                start=(k == 0), stop=(k == KI - 1))
                        yst = stg.tile([128, 512], F32, tag='yst', name=f'ys{h}_{t0}')
                        nc.scalar.copy(out=yst[:, :tn], in_=yps[:, :tn])
                        nc.sync.dma_start(
                            out=yt[h * 128:(h + 1) * 128, t0:t0 + tn],
                            in_=yst[:, :tn])

    _split_sync_waits(nc)
    return nc


def _dispatch(x2, router_w, routing_bias):
    """Host router. Returns per-expert token lists [(token, weight)...]."""
    logits = x2 @ router_w + routing_bias            # [T, 7] fp32
    order = np.argsort(-logits, axis=1, kind='stable')[:, :TOP_K]
    probs = 1.0 / (1.0 + np.exp(-logits))
    rows = np.arange(T)
    s = probs[rows[:, None], order]                  # [T, 2]
    w = s / s.sum(axis=1, keepdims=True)             # renormalized combine weights

    lists = [[] for _ in range(E_ROUTED)]
    for k in range(TOP_K):
        for t, e, wt in zip(rows, order[:, k], w[:, k]):
            lists[e].append((int(t), float(wt)))
    return lists


def _shuffle_gateup(wmat):
    """[H, I] -> [KI, 128(H-part), KH, 128(I-cols)] bf16."""
    return np.ascontiguousarray(
        wmat.reshape(KH, 128, KI, 128).transpose(2, 1, 0, 3).astype(ml_dtypes.bfloat16))


def _shuffle_down(wmat):
    """[I, H] -> [KH(h), 128(I-part), KI(k), 128(H-cols)] bf16."""
    return np.ascontiguousarray(
        wmat.reshape(KI, 128, KH, 128).transpose(2, 1, 0, 3).astype(ml_dtypes.bfloat16))


def kernel(x, router_w, routing_bias, shared_gate, shared_up, shared_down,
           routed_gate, routed_up, routed_down):
    global LAST_RESULT
    x = np.asarray(x, np.float32)
    x2 = x.reshape(T, H)

    lists = _dispatch(x2, np.asarray(router_w, np.float32),
                      np.asarray(routing_bias, np.float32))

    # pieces: split any oversized expert so every piece fits one seg1 slot
    pieces = []                       # (expert_id, [(token, weight)...])
    for e in range(E_ROUTED):
        le = lists[e]
        nsplit = max(1, (len(le) + 2047) // 2048)
        step = (len(le) + nsplit - 1) // nsplit
        for a in range(0, len(le), step):
            pieces.append((e, le[a:a + step]))
    assert len(pieces) <= NCORES, 'expert pieces exceed core count'
    c1 = max(128, max(len(toks) for _, toks in pieces))
    n_spare = NCORES - len(pieces)
    c2 = max(0, -(-(T - n_spare * c1) // NCORES))
    c2 = max(c2, 1)
    C = c1 + c2

    bf = ml_dtypes.bfloat16
    routed_gate = np.asarray(routed_gate, np.float32)
    routed_up = np.asarray(routed_up, np.float32)
    routed_down = np.asarray(routed_down, np.float32)
    gw_s = [_shuffle_gateup(routed_gate[e]) for e in range(E_ROUTED)]
    uw_s = [_shuffle_gateup(routed_up[e]) for e in range(E_ROUTED)]
    dw_s = [_shuffle_down(routed_down[e]) for e in range(E_ROUTED)]
    sg_s = _shuffle_gateup(np.asarray(shared_gate, np.float32))
    su_s = _shuffle_gateup(np.asarray(shared_up, np.float32))
    sd_s = _shuffle_down(np.asarray(shared_down, np.float32))

    # shared-token filler: spare seg1 slots first, then every core's seg2
    shared_ptr = [0]

    def take_shared(n):
        a = shared_ptr[0]
        b = min(T, a + n)
        shared_ptr[0] = b
        return np.arange(a, b)

    in_maps = []
    slot_tok = np.full((NCORES, C), -1, np.int64)
    slot_w = np.ones((NCORES, C), np.float32)
    for c in range(NCORES):
        xgf = np.zeros((C, H), np.float32)
        if c < len(pieces):
            e, toks = pieces[c]
            n = len(toks)
            tok_ids = np.array([t for t, _ in toks], np.int64)
            wts = np.array([wt for _, wt in toks], np.float32)
            xgf[:n] = x2[tok_ids]
            slot_tok[c, :n] = tok_ids
            slot_w[c, :n] = wts
            w1g, w1u, w1d = gw_s[e], uw_s[e], dw_s[e]
        else:
            tok_ids = take_shared(c1)
            n = len(tok_ids)
            xgf[:n] = x2[tok_ids]
            slot_tok[c, :n] = tok_ids
            w1g, w1u, w1d = sg_s, su_s, sd_s
        tok2 = take_shared(c2)
        n2 = len(tok2)
        xgf[c1:c1 + n2] = x2[tok2]
        slot_tok[c, c1:c1 + n2] = tok2
        in_maps.append({
            'xg': np.ascontiguousarray(xgf.T.astype(bf)),
            'g1': w1g, 'u1': w1u, 'd1': w1d,
            'g2': sg_s, 'u2': su_s, 'd2': sd_s,
        })
    assert shared_ptr[0] >= T, 'shared filler did not cover all tokens'

    key = (c1, c2)
    nc = _PROG_CACHE.get(key)
    if nc is None:
        nc = _build_program(key)
        _PROG_CACHE[key] = nc

    res = run_bass_kernel_spmd(nc, in_maps, list(range(NCORES)))
    LAST_RESULT = res

    # host combine: each token's 3 slots (1 shared + 2 routed), weighted sum
    yt_flat = np.concatenate(
        [np.asarray(res.results[c]['yt'], np.float32).T for c in range(NCORES)],
        axis=0)                                            # [8*C, H]
    flat_tok = slot_tok.reshape(-1)
    flat_w = slot_w.reshape(-1)
    valid = np.flatnonzero(flat_tok >= 0)
    order = valid[np.argsort(flat_tok[valid], kind='stable')]
    idx_mat = order.reshape(T, TOP_K + 1)                  # 3 slots per token
    out2 = (yt_flat[idx_mat[:, 0]] * flat_w[idx_mat[:, 0], None]
            + yt_flat[idx_mat[:, 1]] * flat_w[idx_mat[:, 1], None]
            + yt_flat[idx_mat[:, 2]] * flat_w[idx_mat[:, 2], None])
    return out2.reshape(B, S, H).astype(np.float32)


# revision 32
# speedup vs baseline: 1.0820x; 1.0045x over previous
"""DeepSeekMoE (B=2,S=2048,H=1024,I=2816, 7 routed experts top-2 + 1 shared) on 8 trn2 NeuronCores.

Strategy: collective-free unified expert-parallel.
  The shared expert has the same architecture as the routed experts, so every
  unit of work is "one MLP applied to one column" — a column is either a
  (token, routed-expert) slot or a (token, shared) slot.  12288 slot-columns
  total are packed into 8 cores x 2 uniform segments:
    seg1 (cap c1 = max expert load): core c < 7 carries routed expert c's
      entire token list; the remaining seg1 slots are shared-token filler.
    seg2 (cap c2): shared-token filler on every core.
  Slot outputs need no on-chip scaling, no scatter and no ReduceScatter:
  each core returns raw yt = down(silu(xg@G) * (xg@U)) [H, C] and the host
  computes each token's weighted 3-slot sum (shared + top-2 routed, combine
  weights applied there) — O(T*H) flops, ~0.01% of the total, same spirit
  as the host router.

  Device schedule per core (all matmuls bf16, f32 psum):
    1. gate/up: 22 I-chunks; per chunk stream 4 weight tiles (g/u x 2 segs)
       and run K=8-deep matmul groups over ~512-col tiles; silu on the scalar
       engine; h = silu(g)*u written to HT (bf16) by the vector engine.
    2. down: 8 H-chunks; per chunk stream 2 down-weight tiles (2 segs),
       K=22-deep matmul groups over the same column tiles, psum copied out on
       the scalar engine and DMAed to yt [H, C] f32.
  No collectives -> DMA streams freely; weights stream (2 expert sets/core,
  ~35 MB) far below the ~120 GB/s needed to keep pace with the PE.
"""

import math
import os
import sys
import types

import numpy as np
import ml_dtypes

for _p in ('/opt/trn_rl_repo', '/root/.axon_site/_ro/trn_rl_repo'):
    if os.path.isdir(_p) and _p not in sys.path:
        sys.path.append(_p)


def _install_profile_glue():
    """Optional: register the NTFF profile hook so trace=True/BASS_TRACE works
    under axon (the image's antenv lacks axon_hooks). Harmless if unavailable."""
    try:
        import antenv
        if 'antenv.axon_hooks' in sys.modules:
            return
        mod = types.ModuleType('antenv.axon_hooks')
        holder = [None]
        mod.set_axon_ntff_profile_hook = lambda h: holder.__setitem__(0, h)
        mod.get_axon_ntff_profile_hook = lambda: holder[0]
        sys.modules['antenv.axon_hooks'] = mod
        antenv.axon_hooks = mod
        so = '/opt/axon/libaxon_pjrt.so'
        if os.path.exists(so):
            from trn_agent_boot.trn_boot import _ntff_profile_via_ctypes
            hook = _ntff_profile_via_ctypes(so)
            if hook is not None:
                mod.set_axon_ntff_profile_hook(hook)
    except Exception:
        pass


_install_profile_glue()

import concourse.bass as bass
import concourse.mybir as mybir
from concourse.bass_utils import run_bass_kernel_spmd
from concourse.tile import TileContext

B, S, H, I = 2, 2048, 1024, 2816
E_ROUTED = 7
TOP_K = 2
T = B * S                  # 4096 tokens
NCORES = 8
KH = H // 128              # 8 contraction chunks over H (gate/up) = output chunks (down)
KI = I // 128              # 22 contraction chunks over I (down) = output chunks (gate/up)

F32 = mybir.dt.float32
BF16 = mybir.dt.bfloat16

LAST_RESULT = None         # BassKernelResults of the most recent run (for tests)

_PROG_CACHE = {}


def _split_sync_waits(nc, max_waits=1):
    """This container's walrus rejects >1 sync wait per instruction; spill
    extra waits onto same-engine NoOps placed just before the instruction."""
    for f in nc.m.functions:
        for bb in f.blocks:
            new_list = []
            changed = False
            for inst in bb.instructions:
                si = inst.sync_info
                if si is not None and si.on_wait is not None and len(si.on_wait) > max_waits:
                    waits = list(si.on_wait)
                    while len(waits) > max_waits:
                        chunk, waits = waits[:max_waits], waits[max_waits:]
                        nop = mybir.InstNoOp(
                            name=nc.get_next_instruction_name(),
                            engine=inst.engine, bass_nofuse=True,
                            sync_info=mybir.SyncInfo(on_wait=chunk, on_update=[]),
                        )
                        new_list.append(nop)
                    inst.sync_info = mybir.SyncInfo(
                        on_wait=waits, on_update=list(si.on_update or []))
                    changed = True
                new_list.append(inst)
            if changed:
                bb.instructions[:] = new_list


def _even_tiles(offset, total, width=512):
    """Split [offset, offset+total) into near-even tiles of <= width cols."""
    if total <= 0:
        return []
    n = (total + width - 1) // width
    base, rem = divmod(total, n)
    out = []
    c = offset
    for j in range(n):
        tn = base + (1 if j < rem else 0)
        out.append((c, tn))
        c += tn
    return out


WARM_DEFAULT = int(os.environ.get('KWARM', '0'))
STAGE_MODE = os.environ.get('KSTAGE', 'coarse')
XSPLIT = int(os.environ.get('KXSPLIT', '2'))
I0_GATE_FIRST = os.environ.get('KI0', 'gf') == 'gf'


def _build_program(caps):
    """Uniform SPMD program for segment capacities (c1, c2)."""
    c1, c2 = caps
    C = c1 + c2
    tiles1 = _even_tiles(0, c1)
    tiles2 = _even_tiles(c1, c2)

    nc = bass.Bass()
    # one gathered x copy serves gate AND up: the top-2 combine weight is
    # applied by the host to the returned slot outputs, not on-chip.
    xg = nc.declare_dram_parameter('xg', [H, C], BF16, isOutput=False)
    # gate/up weights arrive chunk-shuffled: [KI, 128, KH, 128] so each
    # per-I-chunk stream DMA reads 2KB-contiguous per partition.
    g1 = nc.declare_dram_parameter('g1', [KI, 128, KH, 128], BF16, isOutput=False)
    u1 = nc.declare_dram_parameter('u1', [KI, 128, KH, 128], BF16, isOutput=False)
    g2 = nc.declare_dram_parameter('g2', [KI, 128, KH, 128], BF16, isOutput=False)
    u2 = nc.declare_dram_parameter('u2', [KI, 128, KH, 128], BF16, isOutput=False)
    # down weights shuffled per output H-chunk: [KH, 128, KI, 128]
    d1 = nc.declare_dram_parameter('d1', [KH, 128, KI, 128], BF16, isOutput=False)
    d2 = nc.declare_dram_parameter('d2', [KH, 128, KI, 128], BF16, isOutput=False)
    yt = nc.declare_dram_parameter('yt', [H, C], F32, isOutput=True)

    xg_r = xg.rearrange('(k p) c -> p k c', p=128)
    warm_out = nc.dram_tensor('warm', [128, 128], F32)

    with TileContext(nc) as tc:
        with (
            tc.tile_pool(name='big', bufs=1) as bigp,
            tc.tile_pool(name='wstream', bufs=3) as wsp,
            tc.tile_pool(name='dstream', bufs=3) as dsp,
            tc.tile_pool(name='rtp', bufs=5) as rtp,
            tc.tile_pool(name='stg', bufs=3) as stg,
            tc.tile_pool(name='ps', bufs=8, space='PSUM') as psp,
        ):
            XG = bigp.tile([128, KH, C], BF16, tag='XG')
            HT = bigp.tile([128, KI, C], BF16, tag='HT')

            # ---- 0. PE clock warmup during input staging: the tensor engine
            # ramps to max frequency only after ~3us of continuous execution,
            # so burn dummy matmuls while the first DMAs land.
            WARM = WARM_DEFAULT
            if WARM > 0:
                wu = bigp.tile([128, 256], BF16, tag='wu')
                nc.vector.memset(wu[:, :], 0.0)
                wps = psp.tile([128, 512], F32, tag='ps', name='warm')
                for r in range(WARM):
                    nc.tensor.matmul(wps[:, :256], lhsT=wu[:, :128],
                                     rhs=wu[:, :],
                                     start=(r == 0), stop=(r == WARM - 1))
                wst = stg.tile([128, 512], F32, tag='yst', name='warmst')
                nc.scalar.copy(out=wst[:, :128], in_=wps[:, :128])
                nc.sync.dma_start(out=warm_out[:, :], in_=wst[:, :128])

            # ---- 1. gate/up over 22 I-chunks
            for i in range(KI):
                gch1 = wsp.tile([128, KH, 128], BF16, tag='g1', name=f'g1_{i}')
                nc.sync.dma_start(out=gch1[:, :, :], in_=g1[i, :, :, :])
                gch2 = wsp.tile([128, KH, 128], BF16, tag='g2', name=f'g2_{i}')
                nc.sync.dma_start(out=gch2[:, :, :], in_=g2[i, :, :, :])
                uch1 = wsp.tile([128, KH, 128], BF16, tag='u1', name=f'u1_{i}')
                nc.sync.dma_start(out=uch1[:, :, :], in_=u1[i, :, :, :])
                uch2 = wsp.tile([128, KH, 128], BF16, tag='u2', name=f'u2_{i}')
                nc.sync.dma_start(out=uch2[:, :, :], in_=u2[i, :, :, :])
                if i == 0:
                    for k in range(KH):
                        for (q0, qn) in _even_tiles(0, C, -(-C // XSPLIT)):
                            nc.sync.dma_start(out=XG[:, k, q0:q0 + qn],
                                              in_=xg_r[:, k, q0:q0 + qn])
                work = [(gch1, uch1, t0, tn) for (t0, tn) in tiles1] + \
                       [(gch2, uch2, t0, tn) for (t0, tn) in tiles2]
                gate_first = I0_GATE_FIRST and i == 0
                ats = {}

                def gate_part(gch, t0, tn):
                    gps = psp.tile([128, 512], F32, tag='ps', name=f'g{i}_{t0}')
                    for k in range(KH):
                        nc.tensor.matmul(
                            gps[:, :tn], lhsT=gch[:, k, :],
                            rhs=XG[:, k, t0:t0 + tn],
                            start=(k == 0), stop=(k == KH - 1))
                    at = rtp.tile([128, 512], F32, tag='at', name=f'at{i}_{t0}')
                    nc.scalar.activation(
                        out=at[:, :tn], in_=gps[:, :tn],
                        func=mybir.ActivationFunctionType.Silu)
                    ats[t0] = at

                def up_part(uch, t0, tn):
                    ups = psp.tile([128, 512], F32, tag='ps', name=f'u{i}_{t0}')
                    for k in range(KH):
                        nc.tensor.matmul(
                            ups[:, :tn], lhsT=uch[:, k, :],
                            rhs=XG[:, k, t0:t0 + tn],
                            start=(k == 0), stop=(k == KH - 1))
                    nc.vector.tensor_tensor(
                        out=HT[:, i, t0:t0 + tn],
                        in0=ats[t0][:, :tn], in1=ups[:, :tn],
                        op=mybir.AluOpType.mult)

                if gate_first:
                    for (gch, uch, t0, tn) in work:
                        gate_part(gch, t0, tn)
                    for (gch, uch, t0, tn) in work:
                        up_part(uch, t0, tn)
                else:
                    for (gch, uch, t0, tn) in work:
                        gate_part(gch, t0, tn)
                        up_part(uch, t0, tn)

            # ---- 2. down over 8 H-chunks
            for h in range(KH):
                dch1 = dsp.tile([128, KI, 128], BF16, tag='d1', name=f'd1_{h}')
                nc.sync.dma_start(out=dch1[:, :, :], in_=d1[h, :, :, :])
                dch2 = dsp.tile([128, KI, 128], BF16, tag='d2', name=f'd2_{h}')
                nc.sync.dma_start(out=dch2[:, :, :], in_=d2[h, :, :, :])
                for (dch, tiles) in ((dch1, tiles1), (dch2, tiles2)):
                    for (t0, tn) in tiles:
                        yps = psp.tile([128, 512], F32, tag='ps', name=f'y{h}_{t0}')
                        for k in range(KI):
                            nc.tensor.matmul(
                                yps[:, :tn], lhsT=dch[:, k, :],
                                rhs=HT[:, k, t0:t0 + tn],
                                start=(k == 0), stop=(k == KI - 1))
                        yst = stg.tile([128, 512], F32, tag='yst', name=f'ys{h}_{t0}')
                        nc.scalar.copy(out=yst[:, :tn], in_=yps[:, :tn])
                        nc.sync.dma_start(
                            out=yt[h * 128:(h + 1) * 128, t0:t0 + tn],
                            in_=yst[:, :tn])

    _split_sync_waits(nc)
    return nc


def _dispatch(x2, router_w, routing_bias):
    """Host router. Returns per-expert token lists [(token, weight)...]."""
    logits = x2 @ router_w + routing_bias            # [T, 7] fp32
    order = np.argsort(-logits, axis=1, kind='stable')[:, :TOP_K]
    probs = 1.0 / (1.0 + np.exp(-logits))
    rows = np.arange(T)
    s = probs[rows[:, None], order]                  # [T, 2]
    w = s / s.sum(axis=1, keepdims=True)             # renormalized combine weights

    lists = [[] for _ in range(E_ROUTED)]
    for k in range(TOP_K):
        for t, e, wt in zip(rows, order[:, k], w[:, k]):
            lists[e].append((int(t), float(wt)))
    return lists


def _shuffle_gateup(wmat):
    """[H, I] -> [KI, 128(H-part), KH, 128(I-cols)] bf16."""
    return np.ascontiguousarray(
        wmat.reshape(KH, 128, KI, 128).transpose(2, 1, 0, 3).astype(ml_dtypes.bfloat16))


def _shuffle_down(wmat):
    """[I, H] -> [KH(h), 128(I-part), KI(k), 128(H-cols)] bf16."""
    return np.ascontiguousarray(
        wmat.reshape(KI, 128, KH, 128).transpose(2, 1, 0, 3).astype(ml_dtypes.bfloat16))


def kernel(x, router_w, routing_bias, shared_gate, shared_up, shared_down,
           routed_gate, routed_up, routed_down):
    global LAST_RESULT
    x = np.asarray(x, np.float32)
    x2 = x.reshape(T, H)

    lists = _dispatch(x2, np.asarray(router_w, np.float32),
                      np.asarray(routing_bias, np.float32))

    # pieces: split any oversized expert so every piece fits one seg1 slot
    pieces = []                       # (expert_id, [(token, weight)...])
    for e in range(E_ROUTED):
        le = lists[e]
        nsplit = max(1, (len(le) + 2047) // 2048)
        step = (len(le) + nsplit - 1) // nsplit
        for a in range(0, len(le), step):
            pieces.append((e, le[a:a + step]))
    assert len(pieces) <= NCORES, 'expert pieces exceed core count'
    c1 = max(128, max(len(toks) for _, toks in pieces))
    n_spare = NCORES - len(pieces)
    c2 = max(0, -(-(T - n_spare * c1) // NCORES))
    c2 = max(c2, 1)
    C = c1 + c2

    bf = ml_dtypes.bfloat16
    routed_gate = np.asarray(routed_gate, np.float32)
    routed_up = np.asarray(routed_up, np.float32)
    routed_down = np.asarray(routed_down, np.float32)
    gw_s = [_shuffle_gateup(routed_gate[e]) for e in range(E_ROUTED)]
    uw_s = [_shuffle_gateup(routed_up[e]) for e in range(E_ROUTED)]
    dw_s = [_shuffle_down(routed_down[e]) for e in range(E_ROUTED)]
    sg_s = _shuffle_gateup(np.asarray(shared_gate, np.float32))
    su_s = _shuffle_gateup(np.asarray(shared_up, np.float32))
    sd_s = _shuffle_down(np.asarray(shared_down, np.float32))

    # shared-token filler: spare seg1 slots first, then every core's seg2
    shared_ptr = [0]

    def take_shared(n):
        a = shared_ptr[0]
        b = min(T, a + n)
        shared_ptr[0] = b
        return np.arange(a, b)

    in_maps = []
    slot_tok = np.full((NCORES, C), -1, np.int64)
    slot_w = np.ones((NCORES, C), np.float32)
    for c in range(NCORES):
        xgf = np.zeros((C, H), np.float32)
        if c < len(pieces):
            e, toks = pieces[c]
            n = len(toks)
            tok_ids = np.array([t for t, _ in toks], np.int64)
            wts = np.array([wt for _, wt in toks], np.float32)
            xgf[:n] = x2[tok_ids]
            slot_tok[c, :n] = tok_ids
            slot_w[c, :n] = wts
            w1g, w1u, w1d = gw_s[e], uw_s[e], dw_s[e]
        else:
            tok_ids = take_shared(c1)
            n = len(tok_ids)
            xgf[:n] = x2[tok_ids]
            slot_tok[c, :n] = tok_ids
            w1g, w1u, w1d = sg_s, su_s, sd_s
        tok2 = take_shared(c2)
        n2 = len(tok2)
        xgf[c1:c1 + n2] = x2[tok2]
        slot_tok[c, c1:c1 + n2] = tok2
        in_maps.append({
            'xg': np.ascontiguousarray(xgf.T.astype(bf)),
            'g1': w1g, 'u1': w1u, 'd1': w1d,
            'g2': sg_s, 'u2': su_s, 'd2': sd_s,
        })
    assert shared_ptr[0] >= T, 'shared filler did not cover all tokens'

    key = (c1, c2)
    nc = _PROG_CACHE.get(key)
    if nc is None:
        nc = _build_program(key)
        _PROG_CACHE[key] = nc

    res = run_bass_kernel_spmd(nc, in_maps, list(range(NCORES)))
    LAST_RESULT = res

    # host combine: each token's 3 slots (1 shared + 2 routed), weighted sum
    yt_flat = np.concatenate(
        [np.asarray(res.results[c]['yt'], np.float32).T for c in range(NCORES)],
        axis=0)                                            # [8*C, H]
    flat_tok = slot_tok.reshape(-1)
    flat_w = slot_w.reshape(-1)
    valid = np.flatnonzero(flat_tok >= 0)
    order = valid[np.argsort(flat_tok[valid], kind='stable')]
    idx_mat = order.reshape(T, TOP_K + 1)                  # 3 slots per token
    out2 = (yt_flat[idx_mat[:, 0]] * flat_w[idx_mat[:, 0], None]
            + yt_flat[idx_mat[:, 1]] * flat_w[idx_mat[:, 1], None]
            + yt_flat[idx_mat[:, 2]] * flat_w[idx_mat[:, 2], None])
    return out2.reshape(B, S, H).astype(np.float32)
